# revision 6
# baseline (speedup 1.0000x reference)
"""Trainium2 Bass kernel for a pre-LN transformer block (attention + FFN).

Sharding: 8 cores = (batch b = c//2) x (query-row half = c%2). Each core
computes 1024 query rows end-to-end; K/V for its batch are computed on-core
(duplicated across the 2 cores sharing a batch). No collectives.

Math folds done on host (exact, in f32):
  - LN gains/biases folded into Wq/W1 (gamma row-scales W, beta@W folds into bias)
  - bk dropped (softmax row-shift invariant), bv folded into mix bias
Device computes plain (x-mean)*rstd for both LNs.

Wire format is bf16 for all large tensors (activations, weights, output);
biases and LN scratch stay f32, the residual stream stays f32 on device.
The axon tunnel (~33MB/s) dominates end-to-end latency, so halving bytes
halves latency; bf16 error is ~1e-3 vs the 2e-2 gate.

The PJRT executable is built and jit-compiled ONCE (module-level cache);
per-call work is host prep + transfer + execute. Routing every call
through bass_utils.run_bass_kernel_spmd would rebuild the jit closure
each time (full retrace + NEFF re-import over the tunnel, ~20s/call);
the cached runner binds the same _bass_exec_p primitive that
run_bass_kernel_spmd's axon path (bass2jax.run_bass_via_pjrt) uses, so
the on-device execution is identical. Set _cache["run_kwargs"]
(e.g. trace=True) to route through run_bass_kernel_spmd instead.
"""

import sys

sys.path.insert(0, "/opt/trn_rl_repo")

import numpy as np
import ml_dtypes

import concourse.bass as bass
import concourse.bacc as bacc
import concourse.mybir as mybir
import concourse.tile as tile
from concourse.bass_utils import run_bass_kernel_spmd

F32 = mybir.dt.float32
F32R = mybir.dt.float32r
BF16 = mybir.dt.bfloat16
AF = mybir.ActivationFunctionType
OP = mybir.AluOpType
NPBF16 = ml_dtypes.bfloat16

B, N, D, H = 4, 2048, 512, 8
DH = D // H            # 64
DFF = 4 * D            # 2048
R = 1024               # query rows per core
P = 128
EPS = 1e-5
SCALE = 1.0 / float(np.sqrt(D))

DT = D // P            # 4  Din 128-tiles
RT = R // P            # 8  query-row 128-tiles of this core
KT16 = N // P          # 16 key 128-tiles
QC = R // 512          # 2  query 512-chunks
KC = N // 512          # 4  key 512-chunks
FT = DFF // P          # 16 dff 128-tiles

NCORES = 8

_cache = {}


def _build(gelu_tanh=False):
    nc = bacc.Bacc("TRN2", target_bir_lowering=False, debug=False, num_devices=8)
    dt_ = nc.dram_tensor
    x_d = dt_("x", [R, D], BF16, kind="ExternalInput")
    yt_d = dt_("yt", [D, N], BF16, kind="ExternalInput")
    wq_d = dt_("wq", [D, D], BF16, kind="ExternalInput")
    wk_d = dt_("wk", [D, D], BF16, kind="ExternalInput")
    wv_d = dt_("wv", [D, D], BF16, kind="ExternalInput")
    wmh_d = dt_("wmh", [DH, H, D], BF16, kind="ExternalInput")
    w1_d = dt_("w1", [D, DFF], BF16, kind="ExternalInput")
    w2_d = dt_("w2", [DFF, D], BF16, kind="ExternalInput")
    bq_d = dt_("bq", [D], F32, kind="ExternalInput")
    bm_d = dt_("bm", [D], F32, kind="ExternalInput")
    bb1_d = dt_("bb1", [DFF], F32, kind="ExternalInput")
    bb2_d = dt_("bb2", [D], F32, kind="ExternalInput")
    idm_d = dt_("idm", [P, P], BF16, kind="ExternalInput")
    on1_d = dt_("on1", [P, 1], F32R, kind="ExternalInput")
    on2_d = dt_("on2", [1, P], F32R, kind="ExternalInput")
    onp_d = dt_("onp", [DH + 1, DH], F32R, kind="ExternalInput")
    o_d = dt_("o", [D, R], BF16, kind="ExternalOutput")

    with tile.TileContext(nc) as tc:
        with (
            tc.tile_pool(name="sb", bufs=1) as sb,
            tc.tile_pool(name="scr", bufs=2) as scr,
            tc.tile_pool(name="ps", bufs=4, space="PSUM") as ps,
        ):
            # ---- constants / biases (persist) ----
            ident = sb.tile([P, P], BF16, tag="ident")
            nc.sync.dma_start(ident[:], idm_d.ap())
            ones1x128 = sb.tile([1, P], F32R, tag="o1x128")
            nc.sync.dma_start(ones1x128[:], on2_d.ap())
            onescol = sb.tile([P, 1], F32R, tag="ocol")
            nc.sync.dma_start(onescol[:], on1_d.ap())
            ones2d = sb.tile([DH + 1, DH], F32R, tag="onp")
            nc.sync.dma_start(ones2d[:], onp_d.ap())
            bq_sb = sb.tile([P, DT], F32, tag="bq")
            nc.sync.dma_start(bq_sb[:], bq_d.ap().rearrange("(mt p) -> p mt", p=P))
            bm_sb = sb.tile([P, DT], F32, tag="bm")
            nc.sync.dma_start(bm_sb[:], bm_d.ap().rearrange("(mt p) -> p mt", p=P))
            bb1_sb = sb.tile([P, FT], F32, tag="bb1")
            nc.sync.dma_start(bb1_sb[:], bb1_d.ap().rearrange("(ft p) -> p ft", p=P))
            bb2_sb = sb.tile([P, DT], F32, tag="bb2")
            nc.sync.dma_start(bb2_sb[:], bb2_d.ap().rearrange("(mt p) -> p mt", p=P))
            # residual stream lives whole kernel
            hxt = sb.tile([P, DT, R], F32R, tag="hxt")

            # attention-lifetime pool: closed after mix
            pattn_cm = tc.tile_pool(name="pattn", bufs=1)
            pattn = pattn_cm.__enter__()
            qt128 = pattn.tile([P, DT, R], BF16, tag="qt128")
            kt2 = pattn.tile([P, DT, N], BF16, tag="kt2")
            vaug = pattn.tile([P, KT16, H, DH + 1], BF16, tag="vaug")
            mt_sb = pattn.tile([DH, H, R], BF16, tag="mt")
            wmh_sb = pattn.tile([DH, H, D], BF16, tag="wmh")
            nc.gpsimd.dma_start(wmh_sb[:], wmh_d.ap())

            # ================= phase A: LN0, transposes, Q/K/V =================
            pa1_cm = tc.tile_pool(name="pa1", bufs=1)
            pa1 = pa1_cm.__enter__()
            xr = pa1.tile([P, RT, D], BF16, tag="xr")
            nc.sync.dma_start(xr[:], x_d.ap().rearrange("(rt p) d -> p rt d", p=P))
            xn = xr
            for rt in range(RT):
                sc1 = scr.tile([P, D], F32, tag="lnscr")
                ssum = scr.tile([P, 1], F32, tag="ssum")
                nc.scalar.activation(sc1[:], xr[:, rt], AF.Identity, accum_out=ssum[:])
                sc2 = scr.tile([P, D], F32, tag="lnscr")
                ssq = scr.tile([P, 1], F32, tag="ssq")
                nc.scalar.activation(sc2[:], xr[:, rt], AF.Square, accum_out=ssq[:])
                m = scr.tile([P, 1], F32, tag="m")
                nc.vector.tensor_scalar_mul(m[:], ssum[:], 1.0 / D)
                var = scr.tile([P, 1], F32, tag="var")
                nc.vector.tensor_scalar_mul(var[:], ssq[:], 1.0 / D)
                m2 = scr.tile([P, 1], F32, tag="m2")
                nc.vector.tensor_mul(m2[:], m[:], m[:])
                nc.vector.tensor_sub(var[:], var[:], m2[:])
                nc.vector.tensor_scalar_add(var[:], var[:], EPS)
                std = scr.tile([P, 1], F32, tag="std")
                nc.scalar.activation(std[:], var[:], AF.Sqrt)
                rinv = scr.tile([P, 1], F32, tag="rinv")
                nc.vector.reciprocal(rinv[:], std[:])
                nc.vector.tensor_scalar(
                    xn[:, rt], xr[:, rt], m[:], rinv[:], OP.subtract, OP.mult
                )

            # Xn^T via PE transpose
            pa2_cm = tc.tile_pool(name="pa2", bufs=1)
            pa2 = pa2_cm.__enter__()
            ptp_cm = tc.tile_pool(name="ptp", bufs=2, space="PSUM")
            ptp = ptp_cm.__enter__()
            xnt = pa2.tile([P, DT, R], BF16, tag="xnt")
            wq_sb = pa2.tile([P, DT, D], BF16, tag="wq")
            nc.sync.dma_start(wq_sb[:], wq_d.ap().rearrange("(kt p) m -> p kt m", p=P))
            for rt in range(RT):
                for cb in range(DT):
                    tp = ptp.tile([P, P], BF16, tag="tp")
                    nc.tensor.transpose(tp[:], xn[:, rt, cb * P:(cb + 1) * P], ident[:])
                    nc.vector.tensor_copy(xnt[:, cb, rt * P:(rt + 1) * P], tp[:])

            # Q^T Dout-major, M=128 matmuls straight into qt128
            for mt in range(DT):
                for qc in range(QC):
                    pq = ps.tile([P, 512], F32, tag="mm")
                    for kt in range(DT):
                        nc.tensor.matmul(
                            pq[:],
                            wq_sb[:, kt, mt * P:(mt + 1) * P],
                            xnt[:, kt, qc * 512:(qc + 1) * 512],
                            start=(kt == 0), stop=(kt == DT - 1),
                        )
                    nc.scalar.activation(
                        qt128[:, mt, qc * 512:(qc + 1) * 512], pq[:], AF.Identity,
                        bias=bq_sb[:, mt:mt + 1],
                    )
            ptp_cm.__exit__(None, None, None)
            pa2_cm.__exit__(None, None, None)  # free xnt, wq
            pa1_cm.__exit__(None, None, None)  # free xr

            # K^T head-major and V row-major
            pa3_cm = tc.tile_pool(name="pa3", bufs=1)
            pa3 = pa3_cm.__enter__()
            wk_sb = pa3.tile([P, DT, D], BF16, tag="wk")
            nc.sync.dma_start(wk_sb[:], wk_d.ap().rearrange("(kt p) m -> p kt m", p=P))
            wv_sb = pa3.tile([P, DT, D], BF16, tag="wv")
            nc.sync.dma_start(wv_sb[:], wv_d.ap().rearrange("(kt p) m -> p kt m", p=P))
            nc.vector.memset(vaug[:, :, :, DH:DH + 1], 1.0)

            for khalf in range(2):
                yt_sb = pa3.tile([P, DT, N // 2], BF16, tag="yt", bufs=1)
                nc.sync.dma_start(
                    yt_sb[:],
                    yt_d.ap()[:, khalf * (N // 2):(khalf + 1) * (N // 2)]
                    .rearrange("(kt p) n -> p kt n", p=P),
                )
                for mt in range(DT):
                    for kcl in range(KC // 2):
                        kc = khalf * (KC // 2) + kcl
                        pk = ps.tile([P, 512], F32, tag="mm")
                        for kt in range(DT):
                            nc.tensor.matmul(
                                pk[:],
                                wk_sb[:, kt, mt * P:(mt + 1) * P],
                                yt_sb[:, kt, kcl * 512:(kcl + 1) * 512],
                                start=(kt == 0), stop=(kt == DT - 1),
                            )
                        nc.scalar.copy(kt2[:, mt, kc * 512:(kc + 1) * 512], pk[:])
                for rtl in range(KT16 // 2):
                    rt = khalf * (KT16 // 2) + rtl
                    pv = ps.tile([P, 512], F32, tag="mm")
                    for kt in range(DT):
                        nc.tensor.matmul(
                            pv[:],
                            yt_sb[:, kt, rtl * P:(rtl + 1) * P],
                            wv_sb[:, kt, :],
                            start=(kt == 0), stop=(kt == DT - 1),
                        )
                    nc.scalar.copy(
                        vaug[:, rt, :, 0:DH], pv[:].rearrange("p (h d) -> p h d", h=H)
                    )
            pa3_cm.__exit__(None, None, None)  # free yt, wk, wv

            # ================= phase B: attention =================
            pb_cm = tc.tile_pool(name="pb", bufs=1)
            pb = pb_cm.__enter__()
            pbig_cm = tc.tile_pool(name="pbig", bufs=1, space="PSUM")
            pbig = pbig_cm.__enter__()
            for hp in range(H // 2):
                ats = [pb.tile([P, KT16, R], BF16, tag="at0", bufs=1, name="at0"),
                       pb.tile([P, KT16, R], BF16, tag="at1", bufs=1, name="at1")]
                for kt in range(KT16):
                    pse = pbig.tile([P, R], F32, tag="bigE")
                    pso = pbig.tile([P, R], F32, tag="bigO")
                    for qc in range(QC):
                        nc.tensor.matmul(
                            pse[:, qc * 512:(qc + 1) * 512],
                            kt2[0:DH, hp, kt * P:(kt + 1) * P],
                            qt128[0:DH, hp, qc * 512:(qc + 1) * 512],
                            start=True, stop=True,
                        )
                        nc.tensor.matmul(
                            pso[:, qc * 512:(qc + 1) * 512],
                            kt2[DH:P, hp, kt * P:(kt + 1) * P],
                            qt128[DH:P, hp, qc * 512:(qc + 1) * 512],
                            start=True, stop=True, tile_position=(DH, 0),
                        )
                    nc.scalar.activation(ats[0][:, kt, :], pse[:], AF.Exp, scale=SCALE)
                    nc.scalar.activation(ats[1][:, kt, :], pso[:], AF.Exp, scale=SCALE)
                for par in range(2):
                    h = 2 * hp + par
                    at = ats[par]
                    for qc in range(QC):
                        pav = ps.tile([P, 512], F32, tag="mm")
                        for kt in range(KT16):
                            nc.tensor.matmul(
                                pav[0:DH + 1, :],
                                vaug[:, kt, h, :],
                                at[:, kt, qc * 512:(qc + 1) * 512],
                                start=(kt == 0), stop=(kt == KT16 - 1),
                            )
                        ot_sb = scr.tile([DH, 512], F32, tag="otsb", bufs=2)
                        nc.vector.tensor_copy(ot_sb[:], pav[0:DH, :])
                        rd_sb = scr.tile([DH + 1, 512], F32, tag="rds", bufs=2)
                        nc.vector.reciprocal(rd_sb[DH:DH + 1, :], pav[DH:DH + 1, :])
                        rd_sbr = scr.tile([DH + 1, 512], F32R, tag="rdsr", bufs=2)
                        nc.vector.tensor_copy(rd_sbr[DH:DH + 1, :], rd_sb[DH:DH + 1, :])
                        pbc = ps.tile([DH, 512], F32, tag="mm")
                        nc.tensor.matmul(
                            pbc[:], ones2d[DH:DH + 1, :], rd_sbr[DH:DH + 1, :],
                            start=True, stop=True,
                        )
                        nc.vector.tensor_mul(
                            mt_sb[:, h, qc * 512:(qc + 1) * 512], ot_sb[:], pbc[:]
                        )
            pbig_cm.__exit__(None, None, None)
            pb_cm.__exit__(None, None, None)  # free at

            # ================= phase C: mix + residual =================
            for mt in range(DT):
                for qc in range(QC):
                    pm = ps.tile([P, 512], F32, tag="mm")
                    for h in range(H):
                        nc.tensor.matmul(
                            pm[:],
                            wmh_sb[:, h, mt * P:(mt + 1) * P],
                            mt_sb[:, h, qc * 512:(qc + 1) * 512],
                            start=(h == 0), stop=(h == H - 1),
                        )
                    q = qc * 512
                    nc.vector.tensor_add(
                        hxt[:, mt, q:q + 512], pm[:], qt128[:, mt, q:q + 512]
                    )
                    nc.vector.tensor_scalar_add(
                        hxt[:, mt, q:q + 512], hxt[:, mt, q:q + 512], bm_sb[:, mt:mt + 1]
                    )
            pattn_cm.__exit__(None, None, None)  # free qt128/kt2/vaug/mt/wmh

            # ================= phase D: LN1 (feature-major) + FFN =================
            pd_cm = tc.tile_pool(name="pd", bufs=1)
            pd = pd_cm.__enter__()
            pst_cm = tc.tile_pool(name="pst", bufs=2, space="PSUM")
            pst = pst_cm.__enter__()
            w1_sb = pd.tile([P, DT, DFF], BF16, tag="w1")
            nc.gpsimd.dma_start(w1_sb[:], w1_d.ap().rearrange("(kt p) m -> p kt m", p=P))
            w2_sb = pd.tile([P, FT, D], BF16, tag="w2")
            nc.gpsimd.dma_start(w2_sb[:], w2_d.ap().rearrange("(kt p) m -> p kt m", p=P))

            hxn = pd.tile([P, DT, R], BF16, tag="hxn")
            for qc in range(QC):
                q = qc * 512
                ps_s = pst.tile([1, 512], F32, tag="st")
                for dt in range(DT):
                    nc.tensor.matmul(
                        ps_s[:], onescol[:], hxt[:, dt, q:q + 512],
                        start=(dt == 0), stop=(dt == DT - 1),
                    )
                mean = scr.tile([1, 512], F32, tag="mean", bufs=1)
                nc.vector.tensor_scalar_mul(mean[:], ps_s[:], 1.0 / D)
                ps_q = pst.tile([1, 512], F32, tag="st")
                for dt in range(DT):
                    sqs = scr.tile([P, 512], F32R, tag="sqs", bufs=2)
                    nc.vector.tensor_mul(sqs[:], hxt[:, dt, q:q + 512], hxt[:, dt, q:q + 512])
                    nc.tensor.matmul(
                        ps_q[:], onescol[:], sqs[:],
                        start=(dt == 0), stop=(dt == DT - 1),
                    )
                var = scr.tile([1, 512], F32, tag="lvar", bufs=1)
                nc.vector.tensor_scalar_mul(var[:], ps_q[:], 1.0 / D)
                m2 = scr.tile([1, 512], F32, tag="lm2", bufs=1)
                nc.vector.tensor_mul(m2[:], mean[:], mean[:])
                nc.vector.tensor_sub(var[:], var[:], m2[:])
                nc.vector.tensor_scalar_add(var[:], var[:], EPS)
                std = scr.tile([1, 512], F32, tag="lstd", bufs=1)
                nc.scalar.activation(std[:], var[:], AF.Sqrt)
                rstd32 = scr.tile([1, 512], F32, tag="lrstd32", bufs=1)
                nc.vector.reciprocal(rstd32[:], std[:])
                rstd = scr.tile([1, 512], F32R, tag="lrstd", bufs=1)
                nc.vector.tensor_copy(rstd[:], rstd32[:])
                mrs = scr.tile([1, 512], F32R, tag="lmrs", bufs=1)
                nc.vector.tensor_mul(mrs[:], mean[:], rstd32[:])
                pb_r = ps.tile([P, 512], F32, tag="mm")
                nc.tensor.matmul(pb_r[:], ones1x128[:], rstd[:], start=True, stop=True)
                pb_m = ps.tile([P, 512], F32, tag="mm")
                nc.tensor.matmul(pb_m[:], ones1x128[:], mrs[:], start=True, stop=True)
                for dt in range(DT):
                    nc.vector.tensor_mul(hxn[:, dt, q:q + 512], hxt[:, dt, q:q + 512], pb_r[:])
                    nc.vector.tensor_sub(hxn[:, dt, q:q + 512], hxn[:, dt, q:q + 512], pb_m[:])

            gt = pd.tile([P, FT, R], BF16, tag="gt")
            for ft in range(FT):
                for qc in range(QC):
                    pf = ps.tile([P, 512], F32, tag="mm")
                    for kt in range(DT):
                        nc.tensor.matmul(
                            pf[:],
                            w1_sb[:, kt, ft * P:(ft + 1) * P],
                            hxn[:, kt, qc * 512:(qc + 1) * 512],
                            start=(kt == 0), stop=(kt == DT - 1),
                        )
                    if not gelu_tanh:
                        nc.scalar.activation(
                            gt[:, ft, qc * 512:(qc + 1) * 512], pf[:], AF.Gelu,
                            bias=bb1_sb[:, ft:ft + 1],
                        )
                    else:
                        # sim-only tanh-approx gelu (AF.Gelu unimplemented there)
                        ub = scr.tile([P, 512], F32, tag="gub", bufs=2)
                        nc.scalar.activation(ub[:], pf[:], AF.Identity,
                                             bias=bb1_sb[:, ft:ft + 1])
                        u2 = scr.tile([P, 512], F32, tag="gu2", bufs=2)
                        nc.vector.tensor_mul(u2[:], ub[:], ub[:])
                        nc.vector.tensor_scalar_mul(u2[:], u2[:], 0.044715)
                        nc.vector.tensor_scalar_add(u2[:], u2[:], 1.0)
                        nc.vector.tensor_mul(u2[:], u2[:], ub[:])
                        nc.vector.tensor_scalar_mul(u2[:], u2[:], 0.7978845608028654)
                        th = scr.tile([P, 512], F32, tag="gth", bufs=2)
                        nc.scalar.activation(th[:], u2[:], AF.Tanh)
                        nc.vector.tensor_scalar_add(th[:], th[:], 1.0)
                        nc.vector.tensor_mul(th[:], th[:], ub[:])
                        nc.vector.tensor_scalar_mul(
                            gt[:, ft, qc * 512:(qc + 1) * 512], th[:], 0.5)

            out_sb = pd.tile([P, DT, R], BF16, tag="outsb")
            for mt in range(DT):
                for qc in range(QC):
                    po = ps.tile([P, 512], F32, tag="mm")
                    for kt in range(FT):
                        nc.tensor.matmul(
                            po[:],
                            w2_sb[:, kt, mt * P:(mt + 1) * P],
                            gt[:, kt, qc * 512:(qc + 1) * 512],
                            start=(kt == 0), stop=(kt == FT - 1),
                        )
                    q = qc * 512
                    nc.vector.tensor_add(
                        out_sb[:, mt, q:q + 512], po[:], hxt[:, mt, q:q + 512]
                    )
                    nc.vector.tensor_scalar_add(
                        out_sb[:, mt, q:q + 512], out_sb[:, mt, q:q + 512],
                        bb2_sb[:, mt:mt + 1],
                    )
            nc.gpsimd.dma_start(o_d.ap().rearrange("(mt p) n -> p mt n", p=P), out_sb[:])
            pst_cm.__exit__(None, None, None)
            pd_cm.__exit__(None, None, None)

    nc.compile()
    return nc


def _make_runner(nc, n_cores):
    """Build the reusable jitted SPMD executor for `nc`.

    Mirrors concourse.bass2jax.run_bass_via_pjrt's multi-core branch but
    constructs the jit closure ONCE so repeat calls hit the jit cache
    (run_bass_via_pjrt builds a fresh closure per call, forcing a full
    retrace + executable re-import through the axon tunnel every call).
    """
    import jax
    from jax.sharding import Mesh, PartitionSpec
    from jax.experimental.shard_map import shard_map
    from concourse import bass2jax

    if jax.default_backend() == "axon":
        bass2jax.install_neuronx_cc_hook()

    partition_name = nc.partition_id_tensor.name if nc.partition_id_tensor else None
    in_names, out_names, out_avals, zero_outs = [], [], [], []
    for alloc in nc.m.functions[0].allocations:
        if not isinstance(alloc, mybir.MemoryLocationSet):
            continue
        name = alloc.memorylocations[0].name
        if alloc.kind == "ExternalInput":
            if name != partition_name:
                in_names.append(name)
        elif alloc.kind == "ExternalOutput":
            shape = tuple(alloc.tensor_shape)
            dtype = mybir.dt.np(alloc.dtype)
            out_names.append(name)
            out_avals.append(jax.core.ShapedArray(shape, dtype))
            zero_outs.append(np.zeros((n_cores * shape[0], *shape[1:]), dtype))
    n_params = len(in_names)
    n_outs = len(out_names)
    bind_names = tuple(in_names + out_names + ([partition_name] if partition_name else []))
    donate = tuple(range(n_params, n_params + n_outs))

    def _body(*args):
        operands = list(args)
        if partition_name is not None:
            operands.append(bass2jax.partition_id_tensor())
        outs = bass2jax._bass_exec_p.bind(
            *operands,
            out_avals=tuple(out_avals),
            in_names=bind_names,
            out_names=tuple(out_names),
            lowering_input_output_aliases=(),
            sim_require_finite=True,
            sim_require_nnan=True,
            nc=nc,
        )
        return tuple(outs)

    devices = jax.devices()[:n_cores]
    assert len(devices) == n_cores, f"need {n_cores} devices, have {len(jax.devices())}"
    mesh = Mesh(np.asarray(devices), ("core",))
    in_specs = (PartitionSpec("core"),) * (n_params + n_outs)
    out_specs = (PartitionSpec("core"),) * n_outs
    # cpu (sim) doesn't implement buffer donation; the unaliased
    # jax.buffer_donor annotation trips the sim lowering's check.
    jit_kwargs = {} if jax.default_backend() == "cpu" else dict(donate_argnums=donate)
    fn = jax.jit(
        shard_map(_body, mesh=mesh, in_specs=in_specs, out_specs=out_specs,
                  check_rep=False),
        keep_unused=True, **jit_kwargs,
    )
    return dict(fn=fn, in_names=in_names, out_names=out_names,
                out_avals=out_avals, zero_outs=zero_outs, n_cores=n_cores)


def _run_cached(runner, in_maps):
    n_cores = runner["n_cores"]
    in_names = runner["in_names"]
    per_core = [[np.asarray(m[name]) for name in in_names] for m in in_maps]
    concat_in = [
        np.concatenate([per_core[c][i] for c in range(n_cores)], axis=0)
        for i in range(len(in_names))
    ]
    out_arrs = runner["fn"](*concat_in, *runner["zero_outs"])
    outs = []
    np_out = [np.asarray(a) for a in out_arrs]
    for c in range(n_cores):
        outs.append({
            name: np_out[i].reshape(n_cores, *runner["out_avals"][i].shape)[c]
            for i, name in enumerate(runner["out_names"])
        })
    return outs


def kernel(X, Y, Wq, bq, Wk, bk, Wv, bv, Wm, bm, g0, b0, g1, b1, W1, bb1, W2, bb2,
           **_ignored):
    X = np.asarray(X, dtype=np.float32)
    Y = np.asarray(Y, dtype=np.float32)
    f32 = lambda a: np.asarray(a, dtype=np.float32)
    Wq, bq, Wk, Wv, bv, Wm, bm = map(f32, (Wq, bq, Wk, Wv, bv, Wm, bm))
    g0, b0, g1, b1, W1, bb1, W2, bb2 = map(f32, (g0, b0, g1, b1, W1, bb1, W2, bb2))

    # host-side exact folds (f32), then quantize the wire copies to bf16
    wq = (g0[:, None] * Wq).astype(NPBF16)
    bqv = b0 @ Wq + bq
    wmh = np.ascontiguousarray(
        Wm.reshape(H, DH, D).transpose(1, 0, 2)).astype(NPBF16)
    bmv = bv @ Wm + bm
    w1 = (g1[:, None] * W1).astype(NPBF16)
    bb1v = b1 @ W1 + bb1
    wkb = Wk.astype(NPBF16)
    wvb = Wv.astype(NPBF16)
    w2b = W2.astype(NPBF16)
    xb = X.astype(NPBF16)
    ytb = [Y[b].T.astype(NPBF16) for b in range(B)]
    idm = np.eye(P, dtype=NPBF16)
    on1 = np.ones((P, 1), dtype=np.float32)
    on2 = np.ones((1, P), dtype=np.float32)
    onp = np.ones((DH + 1, DH), dtype=np.float32)

    if "nc" not in _cache:
        _cache["nc"] = _build(gelu_tanh=_cache.get("gelu_tanh", False))
    nc = _cache["nc"]

    in_maps = []
    for c in range(NCORES):
        b, half = c // 2, c % 2
        in_maps.append(dict(
            x=np.ascontiguousarray(xb[b, half * R:(half + 1) * R, :]),
            yt=ytb[b],
            wq=wq, wk=wkb, wv=wvb, wmh=wmh, w1=w1, w2=w2b,
            bq=bqv, bm=bmv, bb1=bb1v, bb2=bb2, idm=idm, on1=on1, on2=on2, onp=onp,
        ))
    if _cache.get("run_kwargs"):
        res = run_bass_kernel_spmd(nc, in_maps, core_ids=list(range(NCORES)),
                                   **_cache["run_kwargs"])
        _cache["last"] = res
        core_outs = res.results
    else:
        if "runner" not in _cache:
            _cache["runner"] = _make_runner(nc, NCORES)
        core_outs = _run_cached(_cache["runner"], in_maps)
        _cache["last"] = None
    out = np.empty((B, N, D), dtype=np.float32)
    for c in range(NCORES):
        b, half = c // 2, c % 2
        out[b, half * R:(half + 1) * R, :] = core_outs[c]["o"].T.astype(np.float32)
    return out


# revision 17
# speedup vs baseline: 2.1329x; 2.1329x over previous
"""Trainium2 Bass kernel for a pre-LN transformer block (attention + FFN).

Sharding: 8 cores = (batch b = c//2) x (query-row half = c%2). Each core
computes 1024 query rows end-to-end; K/V for its batch are computed on-core
(duplicated across the 2 cores sharing a batch). No collectives.

Math folds done on host (exact, in f32):
  - LN gains/biases folded into Wq/W1 (gamma row-scales W, beta@W folds into bias)
  - bk dropped (softmax row-shift invariant), bv folded into mix bias
Device computes plain (x-mean)*rstd for both LNs.

Wire format is bf16 for all large tensors (activations, weights, output);
biases and LN scratch stay f32, the residual stream stays f32 on device.
The axon tunnel (~33MB/s) dominates end-to-end latency, so halving bytes
halves latency; bf16 error is ~1e-3 vs the 2e-2 gate.

The PJRT executable is built and jit-compiled ONCE (module-level cache);
per-call work is host prep + transfer + execute. Routing every call
through bass_utils.run_bass_kernel_spmd would rebuild the jit closure
each time (full retrace + NEFF re-import over the tunnel, ~20s/call);
the cached runner binds the same _bass_exec_p primitive that
run_bass_kernel_spmd's axon path (bass2jax.run_bass_via_pjrt) uses, so
the on-device execution is identical. Set _cache["run_kwargs"]
(e.g. trace=True) to route through run_bass_kernel_spmd instead.
"""

import sys

sys.path.insert(0, "/opt/trn_rl_repo")

import numpy as np
import ml_dtypes

import concourse.bass as bass
import concourse.bacc as bacc
import concourse.mybir as mybir
import concourse.tile as tile
from concourse.bass_utils import run_bass_kernel_spmd

F32 = mybir.dt.float32
F32R = mybir.dt.float32r
BF16 = mybir.dt.bfloat16
AF = mybir.ActivationFunctionType
OP = mybir.AluOpType
NPBF16 = ml_dtypes.bfloat16

B, N, D, H = 4, 2048, 512, 8
DH = D // H            # 64
DFF = 4 * D            # 2048
R = 1024               # query rows per core
P = 128
EPS = 1e-5
SCALE = 1.0 / float(np.sqrt(D))

DT = D // P            # 4  Din 128-tiles
RT = R // P            # 8  query-row 128-tiles of this core
KT16 = N // P          # 16 key 128-tiles
QC = R // 512          # 2  query 512-chunks
KC = N // 512          # 4  key 512-chunks
FT = DFF // P          # 16 dff 128-tiles

NCORES = 8

# packed bf16 weight blob (flat element offsets); each core uploads 1/8 and
# the full blob is AllGather'd on-device over NeuronLink — the axon tunnel
# (~30MB/s) is the bottleneck, so shipping weights once instead of 8x wins.
OFF_WQ = 0
OFF_WK = OFF_WQ + D * D
OFF_WV = OFF_WK + D * D
OFF_WMH = OFF_WV + D * D
OFF_W1 = OFF_WMH + DH * H * D
OFF_W2 = OFF_W1 + D * DFF
WTOT = OFF_W2 + DFF * D          # 3145728
WSH = WTOT // NCORES             # 393216
YTOT = D * N                     # yt blob per batch (1048576)
YSH = YTOT // 2                  # each core of a batch pair uploads half

_cache = {}


def _build(gelu_tanh=False):
    nc = bacc.Bacc("TRN2", target_bir_lowering=False, debug=False, num_devices=8)
    dt_ = nc.dram_tensor
    x_d = dt_("x", [R, D], BF16, kind="ExternalInput")
    ysh_d = dt_("ysh", [YSH], BF16, kind="ExternalInput")
    wsh_d = dt_("wsh", [WSH], BF16, kind="ExternalInput")
    bq_d = dt_("bq", [D], F32, kind="ExternalInput")
    bm_d = dt_("bm", [D], F32, kind="ExternalInput")
    bb1_d = dt_("bb1", [DFF], F32, kind="ExternalInput")
    bb2_d = dt_("bb2", [D], F32, kind="ExternalInput")
    idm_d = dt_("idm", [P, P], BF16, kind="ExternalInput")
    on1_d = dt_("on1", [P, 1], F32R, kind="ExternalInput")
    on2_d = dt_("on2", [1, P], F32R, kind="ExternalInput")
    onp_d = dt_("onp", [DH + 1, DH], F32R, kind="ExternalInput")
    o_d = dt_("o", [D, R], BF16, kind="ExternalOutput")

    with tile.TileContext(nc) as tc:
        with (
            tc.tile_pool(name="sb", bufs=1) as sb,
            tc.tile_pool(name="scr", bufs=2) as scr,
            tc.tile_pool(name="ps", bufs=4, space="PSUM") as ps,
            tc.tile_pool(name="dram", bufs=1, space="DRAM") as dram,
        ):
            # ---- gather the sharded uploads (bounce via DRAM: collectives
            # can't read I/O tensors directly) ----
            win = dram.tile([WSH], BF16, tag="win")
            nc.gpsimd.dma_start(win[:], wsh_d.ap())
            wall = dram.tile([WTOT], BF16, tag="wall")
            nc.gpsimd.collective_compute(
                "AllGather", OP.bypass,
                replica_groups=[list(range(NCORES))],
                ins=[win.opt()], outs=[wall.opt()],
            )
            yin = dram.tile([YSH], BF16, tag="yin")
            nc.gpsimd.dma_start(yin[:], ysh_d.ap())
            ytall = dram.tile([YTOT], BF16, tag="ytall")
            nc.gpsimd.collective_compute(
                "AllGather", OP.bypass,
                replica_groups=[[2 * b, 2 * b + 1] for b in range(B)],
                ins=[yin.opt()], outs=[ytall.opt()],
            )

            # ---- constants / biases (persist) ----
            ident = sb.tile([P, P], BF16, tag="ident")
            nc.sync.dma_start(ident[:], idm_d.ap())
            ones1x128 = sb.tile([1, P], F32R, tag="o1x128")
            nc.sync.dma_start(ones1x128[:], on2_d.ap())
            onescol = sb.tile([P, 1], F32R, tag="ocol")
            nc.sync.dma_start(onescol[:], on1_d.ap())
            ones2d = sb.tile([DH + 1, DH], F32R, tag="onp")
            nc.sync.dma_start(ones2d[:], onp_d.ap())
            bq_sb = sb.tile([P, DT], F32, tag="bq")
            nc.sync.dma_start(bq_sb[:], bq_d.ap().rearrange("(mt p) -> p mt", p=P))
            bm_sb = sb.tile([P, DT], F32, tag="bm")
            nc.sync.dma_start(bm_sb[:], bm_d.ap().rearrange("(mt p) -> p mt", p=P))
            bb1_sb = sb.tile([P, FT], F32, tag="bb1")
            nc.sync.dma_start(bb1_sb[:], bb1_d.ap().rearrange("(ft p) -> p ft", p=P))
            bb2_sb = sb.tile([P, DT], F32, tag="bb2")
            nc.sync.dma_start(bb2_sb[:], bb2_d.ap().rearrange("(mt p) -> p mt", p=P))
            # residual stream lives whole kernel
            hxt = sb.tile([P, DT, R], F32R, tag="hxt")

            # attention-lifetime pool: closed after mix
            pattn_cm = tc.tile_pool(name="pattn", bufs=1)
            pattn = pattn_cm.__enter__()
            qt128 = pattn.tile([P, DT, R], BF16, tag="qt128")
            kt2 = pattn.tile([P, DT, N], BF16, tag="kt2")
            vaug = pattn.tile([P, KT16, H, DH + 1], BF16, tag="vaug")
            mt_sb = pattn.tile([DH, H, R], BF16, tag="mt")
            wmh_sb = pattn.tile([DH, H, D], BF16, tag="wmh")
            nc.gpsimd.dma_start(
                wmh_sb[:],
                wall[OFF_WMH:OFF_W1].rearrange("(d h m) -> d h m", h=H, m=D),
            )

            # ================= phase A: LN0, transposes, Q/K/V =================
            pa1_cm = tc.tile_pool(name="pa1", bufs=1)
            pa1 = pa1_cm.__enter__()
            xr = pa1.tile([P, RT, D], BF16, tag="xr")
            nc.sync.dma_start(xr[:], x_d.ap().rearrange("(rt p) d -> p rt d", p=P))
            xn = xr
            for rt in range(RT):
                sc1 = scr.tile([P, D], F32, tag="lnscr")
                ssum = scr.tile([P, 1], F32, tag="ssum")
                nc.scalar.activation(sc1[:], xr[:, rt], AF.Identity, accum_out=ssum[:])
                sc2 = scr.tile([P, D], F32, tag="lnscr")
                ssq = scr.tile([P, 1], F32, tag="ssq")
                nc.scalar.activation(sc2[:], xr[:, rt], AF.Square, accum_out=ssq[:])
                m = scr.tile([P, 1], F32, tag="m")
                nc.vector.tensor_scalar_mul(m[:], ssum[:], 1.0 / D)
                var = scr.tile([P, 1], F32, tag="var")
                nc.vector.tensor_scalar_mul(var[:], ssq[:], 1.0 / D)
                m2 = scr.tile([P, 1], F32, tag="m2")
                nc.vector.tensor_mul(m2[:], m[:], m[:])
                nc.vector.tensor_sub(var[:], var[:], m2[:])
                nc.vector.tensor_scalar_add(var[:], var[:], EPS)
                std = scr.tile([P, 1], F32, tag="std")
                nc.scalar.activation(std[:], var[:], AF.Sqrt)
                rinv = scr.tile([P, 1], F32, tag="rinv")
                nc.vector.reciprocal(rinv[:], std[:])
                nc.vector.tensor_scalar(
                    xn[:, rt], xr[:, rt], m[:], rinv[:], OP.subtract, OP.mult
                )

            # Xn^T via PE transpose
            pa2_cm = tc.tile_pool(name="pa2", bufs=1)
            pa2 = pa2_cm.__enter__()
            ptp_cm = tc.tile_pool(name="ptp", bufs=2, space="PSUM")
            ptp = ptp_cm.__enter__()
            xnt = pa2.tile([P, DT, R], BF16, tag="xnt")
            wq_sb = pa2.tile([P, DT, D], BF16, tag="wq")
            nc.sync.dma_start(
                wq_sb[:],
                wall[OFF_WQ:OFF_WK].rearrange("(kt p m) -> p kt m", p=P, m=D),
            )
            for rt in range(RT):
                for cb in range(DT):
                    tp = ptp.tile([P, P], BF16, tag="tp")
                    nc.tensor.transpose(tp[:], xn[:, rt, cb * P:(cb + 1) * P], ident[:])
                    nc.vector.tensor_copy(xnt[:, cb, rt * P:(rt + 1) * P], tp[:])

            # Q^T Dout-major, M=128 matmuls straight into qt128
            for mt in range(DT):
                for qc in range(QC):
                    pq = ps.tile([P, 512], F32, tag="mm")
                    for kt in range(DT):
                        nc.tensor.matmul(
                            pq[:],
                            wq_sb[:, kt, mt * P:(mt + 1) * P],
                            xnt[:, kt, qc * 512:(qc + 1) * 512],
                            start=(kt == 0), stop=(kt == DT - 1),
                        )
                    nc.scalar.activation(
                        qt128[:, mt, qc * 512:(qc + 1) * 512], pq[:], AF.Identity,
                        bias=bq_sb[:, mt:mt + 1],
                    )
            ptp_cm.__exit__(None, None, None)
            pa2_cm.__exit__(None, None, None)  # free xnt, wq
            pa1_cm.__exit__(None, None, None)  # free xr

            # K^T head-major and V row-major
            pa3_cm = tc.tile_pool(name="pa3", bufs=1)
            pa3 = pa3_cm.__enter__()
            wk_sb = pa3.tile([P, DT, D], BF16, tag="wk")
            nc.sync.dma_start(
                wk_sb[:],
                wall[OFF_WK:OFF_WV].rearrange("(kt p m) -> p kt m", p=P, m=D),
            )
            wv_sb = pa3.tile([P, DT, D], BF16, tag="wv")
            nc.sync.dma_start(
                wv_sb[:],
                wall[OFF_WV:OFF_WMH].rearrange("(kt p m) -> p kt m", p=P, m=D),
            )
            nc.vector.memset(vaug[:, :, :, DH:DH + 1], 1.0)

            for khalf in range(2):
                yt_sb = pa3.tile([P, DT, N // 2], BF16, tag="yt", bufs=1)
                nc.sync.dma_start(
                    yt_sb[:],
                    ytall[:].rearrange("(kt p n) -> p kt n", p=P, n=N)
                    [:, :, khalf * (N // 2):(khalf + 1) * (N // 2)],
                )
                for mt in range(DT):
                    for kcl in range(KC // 2):
                        kc = khalf * (KC // 2) + kcl
                        pk = ps.tile([P, 512], F32, tag="mm")
                        for kt in range(DT):
                            nc.tensor.matmul(
                                pk[:],
                                wk_sb[:, kt, mt * P:(mt + 1) * P],
                                yt_sb[:, kt, kcl * 512:(kcl + 1) * 512],
                                start=(kt == 0), stop=(kt == DT - 1),
                            )
                        nc.scalar.copy(kt2[:, mt, kc * 512:(kc + 1) * 512], pk[:])
                for rtl in range(KT16 // 2):
                    rt = khalf * (KT16 // 2) + rtl
                    pv = ps.tile([P, 512], F32, tag="mm")
                    for kt in range(DT):
                        nc.tensor.matmul(
                            pv[:],
                            yt_sb[:, kt, rtl * P:(rtl + 1) * P],
                            wv_sb[:, kt, :],
                            start=(kt == 0), stop=(kt == DT - 1),
                        )
                    nc.scalar.copy(
                        vaug[:, rt, :, 0:DH], pv[:].rearrange("p (h d) -> p h d", h=H)
                    )
            pa3_cm.__exit__(None, None, None)  # free yt, wk, wv

            # ================= phase B: attention =================
            pb_cm = tc.tile_pool(name="pb", bufs=1)
            pb = pb_cm.__enter__()
            pbig_cm = tc.tile_pool(name="pbig", bufs=1, space="PSUM")
            pbig = pbig_cm.__enter__()
            for hp in range(H // 2):
                ats = [pb.tile([P, KT16, R], BF16, tag="at0", bufs=1, name="at0"),
                       pb.tile([P, KT16, R], BF16, tag="at1", bufs=1, name="at1")]
                for kt in range(KT16):
                    pse = pbig.tile([P, R], F32, tag="bigE")
                    pso = pbig.tile([P, R], F32, tag="bigO")
                    for qc in range(QC):
                        nc.tensor.matmul(
                            pse[:, qc * 512:(qc + 1) * 512],
                            kt2[0:DH, hp, kt * P:(kt + 1) * P],
                            qt128[0:DH, hp, qc * 512:(qc + 1) * 512],
                            start=True, stop=True,
                        )
                        nc.tensor.matmul(
                            pso[:, qc * 512:(qc + 1) * 512],
                            kt2[DH:P, hp, kt * P:(kt + 1) * P],
                            qt128[DH:P, hp, qc * 512:(qc + 1) * 512],
                            start=True, stop=True, tile_position=(DH, 0),
                        )
                    nc.scalar.activation(ats[0][:, kt, :], pse[:], AF.Exp, scale=SCALE)
                    nc.scalar.activation(ats[1][:, kt, :], pso[:], AF.Exp, scale=SCALE)
                for par in range(2):
                    h = 2 * hp + par
                    at = ats[par]
                    for qc in range(QC):
                        pav = ps.tile([P, 512], F32, tag="mm")
                        for kt in range(KT16):
                            nc.tensor.matmul(
                                pav[0:DH + 1, :],
                                vaug[:, kt, h, :],
                                at[:, kt, qc * 512:(qc + 1) * 512],
                                start=(kt == 0), stop=(kt == KT16 - 1),
                            )
                        ot_sb = scr.tile([DH, 512], F32, tag="otsb", bufs=2)
                        nc.vector.tensor_copy(ot_sb[:], pav[0:DH, :])
                        rd_sb = scr.tile([DH + 1, 512], F32, tag="rds", bufs=2)
                        nc.vector.reciprocal(rd_sb[DH:DH + 1, :], pav[DH:DH + 1, :])
                        rd_sbr = scr.tile([DH + 1, 512], F32R, tag="rdsr", bufs=2)
                        nc.vector.tensor_copy(rd_sbr[DH:DH + 1, :], rd_sb[DH:DH + 1, :])
                        pbc = ps.tile([DH, 512], F32, tag="mm")
                        nc.tensor.matmul(
                            pbc[:], ones2d[DH:DH + 1, :], rd_sbr[DH:DH + 1, :],
                            start=True, stop=True,
                        )
                        nc.vector.tensor_mul(
                            mt_sb[:, h, qc * 512:(qc + 1) * 512], ot_sb[:], pbc[:]
                        )
            pbig_cm.__exit__(None, None, None)
            pb_cm.__exit__(None, None, None)  # free at

            # ================= phase C: mix + residual =================
            for mt in range(DT):
                for qc in range(QC):
                    pm = ps.tile([P, 512], F32, tag="mm")
                    for h in range(H):
                        nc.tensor.matmul(
                            pm[:],
                            wmh_sb[:, h, mt * P:(mt + 1) * P],
                            mt_sb[:, h, qc * 512:(qc + 1) * 512],
                            start=(h == 0), stop=(h == H - 1),
                        )
                    q = qc * 512
                    nc.vector.tensor_add(
                        hxt[:, mt, q:q + 512], pm[:], qt128[:, mt, q:q + 512]
                    )
                    nc.vector.tensor_scalar_add(
                        hxt[:, mt, q:q + 512], hxt[:, mt, q:q + 512], bm_sb[:, mt:mt + 1]
                    )
            pattn_cm.__exit__(None, None, None)  # free qt128/kt2/vaug/mt/wmh

            # ================= phase D: LN1 (feature-major) + FFN =================
            pd_cm = tc.tile_pool(name="pd", bufs=1)
            pd = pd_cm.__enter__()
            pst_cm = tc.tile_pool(name="pst", bufs=2, space="PSUM")
            pst = pst_cm.__enter__()
            w1_sb = pd.tile([P, DT, DFF], BF16, tag="w1")
            nc.gpsimd.dma_start(
                w1_sb[:],
                wall[OFF_W1:OFF_W2].rearrange("(kt p m) -> p kt m", p=P, m=DFF),
            )
            w2_sb = pd.tile([P, FT, D], BF16, tag="w2")
            nc.gpsimd.dma_start(
                w2_sb[:],
                wall[OFF_W2:WTOT].rearrange("(kt p m) -> p kt m", p=P, m=D),
            )

            hxn = pd.tile([P, DT, R], BF16, tag="hxn")
            for qc in range(QC):
                q = qc * 512
                ps_s = pst.tile([1, 512], F32, tag="st")
                for dt in range(DT):
                    nc.tensor.matmul(
                        ps_s[:], onescol[:], hxt[:, dt, q:q + 512],
                        start=(dt == 0), stop=(dt == DT - 1),
                    )
                mean = scr.tile([1, 512], F32, tag="mean", bufs=1)
                nc.vector.tensor_scalar_mul(mean[:], ps_s[:], 1.0 / D)
                ps_q = pst.tile([1, 512], F32, tag="st")
                for dt in range(DT):
                    sqs = scr.tile([P, 512], F32R, tag="sqs", bufs=2)
                    nc.vector.tensor_mul(sqs[:], hxt[:, dt, q:q + 512], hxt[:, dt, q:q + 512])
                    nc.tensor.matmul(
                        ps_q[:], onescol[:], sqs[:],
                        start=(dt == 0), stop=(dt == DT - 1),
                    )
                var = scr.tile([1, 512], F32, tag="lvar", bufs=1)
                nc.vector.tensor_scalar_mul(var[:], ps_q[:], 1.0 / D)
                m2 = scr.tile([1, 512], F32, tag="lm2", bufs=1)
                nc.vector.tensor_mul(m2[:], mean[:], mean[:])
                nc.vector.tensor_sub(var[:], var[:], m2[:])
                nc.vector.tensor_scalar_add(var[:], var[:], EPS)
                std = scr.tile([1, 512], F32, tag="lstd", bufs=1)
                nc.scalar.activation(std[:], var[:], AF.Sqrt)
                rstd32 = scr.tile([1, 512], F32, tag="lrstd32", bufs=1)
                nc.vector.reciprocal(rstd32[:], std[:])
                rstd = scr.tile([1, 512], F32R, tag="lrstd", bufs=1)
                nc.vector.tensor_copy(rstd[:], rstd32[:])
                mrs = scr.tile([1, 512], F32R, tag="lmrs", bufs=1)
                nc.vector.tensor_mul(mrs[:], mean[:], rstd32[:])
                pb_r = ps.tile([P, 512], F32, tag="mm")
                nc.tensor.matmul(pb_r[:], ones1x128[:], rstd[:], start=True, stop=True)
                pb_m = ps.tile([P, 512], F32, tag="mm")
                nc.tensor.matmul(pb_m[:], ones1x128[:], mrs[:], start=True, stop=True)
                for dt in range(DT):
                    nc.vector.tensor_mul(hxn[:, dt, q:q + 512], hxt[:, dt, q:q + 512], pb_r[:])
                    nc.vector.tensor_sub(hxn[:, dt, q:q + 512], hxn[:, dt, q:q + 512], pb_m[:])

            gt = pd.tile([P, FT, R], BF16, tag="gt")
            for ft in range(FT):
                for qc in range(QC):
                    pf = ps.tile([P, 512], F32, tag="mm")
                    for kt in range(DT):
                        nc.tensor.matmul(
                            pf[:],
                            w1_sb[:, kt, ft * P:(ft + 1) * P],
                            hxn[:, kt, qc * 512:(qc + 1) * 512],
                            start=(kt == 0), stop=(kt == DT - 1),
                        )
                    if not gelu_tanh:
                        nc.scalar.activation(
                            gt[:, ft, qc * 512:(qc + 1) * 512], pf[:], AF.Gelu,
                            bias=bb1_sb[:, ft:ft + 1],
                        )
                    else:
                        # sim-only tanh-approx gelu (AF.Gelu unimplemented there)
                        ub = scr.tile([P, 512], F32, tag="gub", bufs=2)
                        nc.scalar.activation(ub[:], pf[:], AF.Identity,
                                             bias=bb1_sb[:, ft:ft + 1])
                        u2 = scr.tile([P, 512], F32, tag="gu2", bufs=2)
                        nc.vector.tensor_mul(u2[:], ub[:], ub[:])
                        nc.vector.tensor_scalar_mul(u2[:], u2[:], 0.044715)
                        nc.vector.tensor_scalar_add(u2[:], u2[:], 1.0)
                        nc.vector.tensor_mul(u2[:], u2[:], ub[:])
                        nc.vector.tensor_scalar_mul(u2[:], u2[:], 0.7978845608028654)
                        th = scr.tile([P, 512], F32, tag="gth", bufs=2)
                        nc.scalar.activation(th[:], u2[:], AF.Tanh)
                        nc.vector.tensor_scalar_add(th[:], th[:], 1.0)
                        nc.vector.tensor_mul(th[:], th[:], ub[:])
                        nc.vector.tensor_scalar_mul(
                            gt[:, ft, qc * 512:(qc + 1) * 512], th[:], 0.5)

            out_sb = pd.tile([P, DT, R], BF16, tag="outsb")
            for mt in range(DT):
                for qc in range(QC):
                    po = ps.tile([P, 512], F32, tag="mm")
                    for kt in range(FT):
                        nc.tensor.matmul(
                            po[:],
                            w2_sb[:, kt, mt * P:(mt + 1) * P],
                            gt[:, kt, qc * 512:(qc + 1) * 512],
                            start=(kt == 0), stop=(kt == FT - 1),
                        )
                    q = qc * 512
                    nc.vector.tensor_add(
                        out_sb[:, mt, q:q + 512], po[:], hxt[:, mt, q:q + 512]
                    )
                    nc.vector.tensor_scalar_add(
                        out_sb[:, mt, q:q + 512], out_sb[:, mt, q:q + 512],
                        bb2_sb[:, mt:mt + 1],
                    )
            nc.gpsimd.dma_start(o_d.ap().rearrange("(mt p) n -> p mt n", p=P), out_sb[:])
            pst_cm.__exit__(None, None, None)
            pd_cm.__exit__(None, None, None)

    nc.compile()
    return nc


def _make_runner(nc, n_cores):
    """Build the reusable jitted SPMD executor for `nc`.

    Mirrors concourse.bass2jax.run_bass_via_pjrt's multi-core branch but
    constructs the jit closure ONCE so repeat calls hit the jit cache
    (run_bass_via_pjrt builds a fresh closure per call, forcing a full
    retrace + executable re-import through the axon tunnel every call).
    """
    import jax
    from jax.sharding import Mesh, PartitionSpec
    from jax.experimental.shard_map import shard_map
    from concourse import bass2jax

    if jax.default_backend() == "axon":
        bass2jax.install_neuronx_cc_hook()

    partition_name = nc.partition_id_tensor.name if nc.partition_id_tensor else None
    in_names, out_names, out_avals, zero_outs = [], [], [], []
    for alloc in nc.m.functions[0].allocations:
        if not isinstance(alloc, mybir.MemoryLocationSet):
            continue
        name = alloc.memorylocations[0].name
        if alloc.kind == "ExternalInput":
            if name != partition_name:
                in_names.append(name)
        elif alloc.kind == "ExternalOutput":
            shape = tuple(alloc.tensor_shape)
            dtype = mybir.dt.np(alloc.dtype)
            out_names.append(name)
            out_avals.append(jax.core.ShapedArray(shape, dtype))
            zero_outs.append(np.zeros((n_cores * shape[0], *shape[1:]), dtype))
    n_params = len(in_names)
    n_outs = len(out_names)
    bind_names = tuple(in_names + out_names + ([partition_name] if partition_name else []))
    donate = tuple(range(n_params, n_params + n_outs))

    def _body(*args):
        operands = list(args)
        if partition_name is not None:
            operands.append(bass2jax.partition_id_tensor())
        outs = bass2jax._bass_exec_p.bind(
            *operands,
            out_avals=tuple(out_avals),
            in_names=bind_names,
            out_names=tuple(out_names),
            lowering_input_output_aliases=(),
            sim_require_finite=True,
            sim_require_nnan=True,
            nc=nc,
        )
        return tuple(outs)

    devices = jax.devices()[:n_cores]
    assert len(devices) == n_cores, f"need {n_cores} devices, have {len(jax.devices())}"
    mesh = Mesh(np.asarray(devices), ("core",))
    in_specs = (PartitionSpec("core"),) * (n_params + n_outs)
    out_specs = (PartitionSpec("core"),) * n_outs
    # cpu (sim) doesn't implement buffer donation; the unaliased
    # jax.buffer_donor annotation trips the sim lowering's check.
    jit_kwargs = {} if jax.default_backend() == "cpu" else dict(donate_argnums=donate)
    fn = jax.jit(
        shard_map(_body, mesh=mesh, in_specs=in_specs, out_specs=out_specs,
                  check_rep=False),
        keep_unused=True, **jit_kwargs,
    )
    return dict(fn=fn, in_names=in_names, out_names=out_names,
                out_avals=out_avals, zero_outs=zero_outs, n_cores=n_cores)


def _run_cached(runner, in_maps):
    import time

    n_cores = runner["n_cores"]
    in_names = runner["in_names"]
    per_core = [[np.asarray(m[name]) for name in in_names] for m in in_maps]
    concat_in = [
        np.concatenate([per_core[c][i] for c in range(n_cores)], axis=0)
        for i in range(len(in_names))
    ]
    # The terminal-side worker takes minutes to restart after a previous
    # process's teardown; fresh connections see transient
    # NRT_EXEC_UNIT_UNRECOVERABLE / UNAVAILABLE until it's back. Retry
    # patiently — this only ever triggers on the first call of a process.
    last_err = None
    for attempt in range(30):
        try:
            out_arrs = runner["fn"](*concat_in, *runner["zero_outs"])
            break
        except Exception as e:  # noqa: BLE001
            last_err = e
            msg = f"{type(e).__name__}: {e}"
            transient = ("UNRECOVERABLE" in msg or "UNAVAILABLE" in msg
                         or "NRT_" in msg or "PassThrough" in msg)
            if not transient or attempt == 29:
                raise
            time.sleep(40)
    outs = []
    np_out = [np.asarray(a) for a in out_arrs]
    for c in range(n_cores):
        outs.append({
            name: np_out[i].reshape(n_cores, *runner["out_avals"][i].shape)[c]
            for i, name in enumerate(runner["out_names"])
        })
    return outs


def kernel(X, Y, Wq, bq, Wk, bk, Wv, bv, Wm, bm, g0, b0, g1, b1, W1, bb1, W2, bb2,
           **_ignored):
    X = np.asarray(X, dtype=np.float32)
    Y = np.asarray(Y, dtype=np.float32)
    f32 = lambda a: np.asarray(a, dtype=np.float32)
    Wq, bq, Wk, Wv, bv, Wm, bm = map(f32, (Wq, bq, Wk, Wv, bv, Wm, bm))
    g0, b0, g1, b1, W1, bb1, W2, bb2 = map(f32, (g0, b0, g1, b1, W1, bb1, W2, bb2))

    # host-side exact folds (f32), then quantize the wire copies to bf16,
    # packed into one flat blob that's sharded 1/8-per-core and AllGather'd
    # on device.
    bqv = b0 @ Wq + bq
    bmv = bv @ Wm + bm
    bb1v = b1 @ W1 + bb1
    wblob = np.empty(WTOT, dtype=NPBF16)
    wblob[OFF_WQ:OFF_WK] = (g0[:, None] * Wq).astype(NPBF16).ravel()
    wblob[OFF_WK:OFF_WV] = Wk.astype(NPBF16).ravel()
    wblob[OFF_WV:OFF_WMH] = Wv.astype(NPBF16).ravel()
    wblob[OFF_WMH:OFF_W1] = np.ascontiguousarray(
        Wm.reshape(H, DH, D).transpose(1, 0, 2)).astype(NPBF16).ravel()
    wblob[OFF_W1:OFF_W2] = (g1[:, None] * W1).astype(NPBF16).ravel()
    wblob[OFF_W2:WTOT] = W2.astype(NPBF16).ravel()
    xb = X.astype(NPBF16)
    ytb = [Y[b].T.astype(NPBF16).ravel() for b in range(B)]
    idm = np.eye(P, dtype=NPBF16)
    on1 = np.ones((P, 1), dtype=np.float32)
    on2 = np.ones((1, P), dtype=np.float32)
    onp = np.ones((DH + 1, DH), dtype=np.float32)

    if "nc" not in _cache:
        _cache["nc"] = _build(gelu_tanh=_cache.get("gelu_tanh", False))
    nc = _cache["nc"]

    in_maps = []
    for c in range(NCORES):
        b, half = c // 2, c % 2
        in_maps.append(dict(
            x=np.ascontiguousarray(xb[b, half * R:(half + 1) * R, :]),
            ysh=ytb[b][half * YSH:(half + 1) * YSH],
            wsh=wblob[c * WSH:(c + 1) * WSH],
            bq=bqv, bm=bmv, bb1=bb1v, bb2=bb2, idm=idm, on1=on1, on2=on2, onp=onp,
        ))
    if _cache.get("run_kwargs"):
        res = run_bass_kernel_spmd(nc, in_maps, core_ids=list(range(NCORES)),
                                   **_cache["run_kwargs"])
        _cache["last"] = res
        core_outs = res.results
    else:
        if "runner" not in _cache:
            _cache["runner"] = _make_runner(nc, NCORES)
        core_outs = _run_cached(_cache["runner"], in_maps)
        _cache["last"] = None
    out = np.empty((B, N, D), dtype=np.float32)
    for c in range(NCORES):
        b, half = c // 2, c % 2
        out[b, half * R:(half + 1) * R, :] = core_outs[c]["o"].T.astype(np.float32)
    return out


# revision 20
# speedup vs baseline: 2.4763x; 1.1610x over previous
"""Trainium2 Bass kernel for a pre-LN transformer block (attention + FFN).

Sharding: 8 cores = (batch b = c//2) x (query-row half = c%2). Each core
computes 1024 query rows end-to-end; K/V for its batch are computed on-core
(duplicated across the 2 cores sharing a batch). No collectives.

Math folds done on host (exact, in f32):
  - LN gains/biases folded into Wq/W1 (gamma row-scales W, beta@W folds into bias)
  - bk dropped (softmax row-shift invariant), bv folded into mix bias
Device computes plain (x-mean)*rstd for both LNs.

Wire format is bf16 for all large tensors (activations, weights, output);
biases and LN scratch stay f32, the residual stream stays f32 on device.
The axon tunnel (~33MB/s) dominates end-to-end latency, so halving bytes
halves latency; bf16 error is ~1e-3 vs the 2e-2 gate.

The PJRT executable is built and jit-compiled ONCE (module-level cache);
per-call work is host prep + transfer + execute. Routing every call
through bass_utils.run_bass_kernel_spmd would rebuild the jit closure
each time (full retrace + NEFF re-import over the tunnel, ~20s/call);
the cached runner binds the same _bass_exec_p primitive that
run_bass_kernel_spmd's axon path (bass2jax.run_bass_via_pjrt) uses, so
the on-device execution is identical. Set _cache["run_kwargs"]
(e.g. trace=True) to route through run_bass_kernel_spmd instead.
"""

import sys

sys.path.insert(0, "/opt/trn_rl_repo")

import numpy as np
import ml_dtypes

import concourse.bass as bass
import concourse.bacc as bacc
import concourse.mybir as mybir
import concourse.tile as tile
from concourse.bass_utils import run_bass_kernel_spmd

F32 = mybir.dt.float32
F32R = mybir.dt.float32r
BF16 = mybir.dt.bfloat16
AF = mybir.ActivationFunctionType
OP = mybir.AluOpType
NPBF16 = ml_dtypes.bfloat16

B, N, D, H = 4, 2048, 512, 8
DH = D // H            # 64
DFF = 4 * D            # 2048
R = 1024               # query rows per core
P = 128
EPS = 1e-5
SCALE = 1.0 / float(np.sqrt(D))

DT = D // P            # 4  Din 128-tiles
RT = R // P            # 8  query-row 128-tiles of this core
KT16 = N // P          # 16 key 128-tiles
QC = R // 512          # 2  query 512-chunks
KC = N // 512          # 4  key 512-chunks
FT = DFF // P          # 16 dff 128-tiles

NCORES = 8

# packed bf16 weight blob (flat element offsets); each core uploads 1/8 and
# the full blob is AllGather'd on-device over NeuronLink — the axon tunnel
# (~30MB/s) is the bottleneck, so shipping weights once instead of 8x wins.
OFF_WQ = 0
OFF_WK = OFF_WQ + D * D
OFF_WV = OFF_WK + D * D
OFF_WMH = OFF_WV + D * D
OFF_W1 = OFF_WMH + DH * H * D
OFF_W2 = OFF_W1 + D * DFF
WTOT = OFF_W2 + DFF * D          # 3145728
WSH = WTOT // NCORES             # 393216
YTOT = D * N                     # yt blob per batch (1048576)
YSH = YTOT // 2                  # each core of a batch pair uploads half

_cache = {}


def _build(gelu_tanh=False):
    nc = bacc.Bacc("TRN2", target_bir_lowering=False, debug=False, num_devices=8)
    dt_ = nc.dram_tensor
    x_d = dt_("x", [R, D], BF16, kind="ExternalInput")
    ysh_d = dt_("ysh", [YSH], BF16, kind="ExternalInput")
    wsh_d = dt_("wsh", [WSH], BF16, kind="ExternalInput")
    bq_d = dt_("bq", [D], F32, kind="ExternalInput")
    bm_d = dt_("bm", [D], F32, kind="ExternalInput")
    bb1_d = dt_("bb1", [DFF], F32, kind="ExternalInput")
    bb2_d = dt_("bb2", [D], F32, kind="ExternalInput")
    idm_d = dt_("idm", [P, P], BF16, kind="ExternalInput")
    on1_d = dt_("on1", [P, 1], F32R, kind="ExternalInput")
    on2_d = dt_("on2", [1, P], F32R, kind="ExternalInput")
    onp_d = dt_("onp", [DH + 1, DH], F32R, kind="ExternalInput")
    o_d = dt_("o", [D, R], BF16, kind="ExternalOutput")

    with tile.TileContext(nc) as tc:
        with (
            tc.tile_pool(name="sb", bufs=1) as sb,
            tc.tile_pool(name="scr", bufs=2) as scr,
            tc.tile_pool(name="ps", bufs=4, space="PSUM") as ps,
            tc.tile_pool(name="dram", bufs=1, space="DRAM") as dram,
        ):
            # ---- gather the sharded uploads (bounce via DRAM: collectives
            # can't read I/O tensors directly) ----
            win = dram.tile([WSH], BF16, tag="win")
            nc.gpsimd.dma_start(win[:], wsh_d.ap())
            wall = dram.tile([WTOT], BF16, tag="wall")
            nc.gpsimd.collective_compute(
                "AllGather", OP.bypass,
                replica_groups=[list(range(NCORES))],
                ins=[win.opt()], outs=[wall.opt()],
            )
            yin = dram.tile([YSH], BF16, tag="yin")
            nc.gpsimd.dma_start(yin[:], ysh_d.ap())
            ytall = dram.tile([YTOT], BF16, tag="ytall")
            nc.gpsimd.collective_compute(
                "AllGather", OP.bypass,
                replica_groups=[[2 * b, 2 * b + 1] for b in range(B)],
                ins=[yin.opt()], outs=[ytall.opt()],
            )

            # ---- constants / biases (persist) ----
            ident = sb.tile([P, P], BF16, tag="ident")
            nc.sync.dma_start(ident[:], idm_d.ap())
            ones1x128 = sb.tile([1, P], F32R, tag="o1x128")
            nc.sync.dma_start(ones1x128[:], on2_d.ap())
            onescol = sb.tile([P, 1], F32R, tag="ocol")
            nc.sync.dma_start(onescol[:], on1_d.ap())
            ones2d = sb.tile([DH + 1, DH], F32R, tag="onp")
            nc.sync.dma_start(ones2d[:], onp_d.ap())
            bq_sb = sb.tile([P, DT], F32, tag="bq")
            nc.sync.dma_start(bq_sb[:], bq_d.ap().rearrange("(mt p) -> p mt", p=P))
            bm_sb = sb.tile([P, DT], F32, tag="bm")
            nc.sync.dma_start(bm_sb[:], bm_d.ap().rearrange("(mt p) -> p mt", p=P))
            bb1_sb = sb.tile([P, FT], F32, tag="bb1")
            nc.sync.dma_start(bb1_sb[:], bb1_d.ap().rearrange("(ft p) -> p ft", p=P))
            bb2_sb = sb.tile([P, DT], F32, tag="bb2")
            nc.sync.dma_start(bb2_sb[:], bb2_d.ap().rearrange("(mt p) -> p mt", p=P))
            # residual stream lives whole kernel
            hxt = sb.tile([P, DT, R], F32R, tag="hxt")

            # attention-lifetime pool: closed after mix
            pattn_cm = tc.tile_pool(name="pattn", bufs=1)
            pattn = pattn_cm.__enter__()
            qt128 = pattn.tile([P, DT, R], BF16, tag="qt128")
            kt2 = pattn.tile([P, DT, N], BF16, tag="kt2")
            vaug = pattn.tile([P, KT16, H, DH + 1], BF16, tag="vaug")
            mt_sb = pattn.tile([DH, H, R], BF16, tag="mt")
            wmh_sb = pattn.tile([DH, H, D], BF16, tag="wmh")
            nc.gpsimd.dma_start(
                wmh_sb[:],
                wall[OFF_WMH:OFF_W1].rearrange("(d h m) -> d h m", h=H, m=D),
            )

            # ================= phase A: LN0, transposes, Q/K/V =================
            pa1_cm = tc.tile_pool(name="pa1", bufs=1)
            pa1 = pa1_cm.__enter__()
            xr = pa1.tile([P, RT, D], BF16, tag="xr")
            nc.sync.dma_start(xr[:], x_d.ap().rearrange("(rt p) d -> p rt d", p=P))
            xn = xr
            for rt in range(RT):
                sc1 = scr.tile([P, D], F32, tag="lnscr")
                ssum = scr.tile([P, 1], F32, tag="ssum")
                nc.scalar.activation(sc1[:], xr[:, rt], AF.Identity, accum_out=ssum[:])
                sc2 = scr.tile([P, D], F32, tag="lnscr")
                ssq = scr.tile([P, 1], F32, tag="ssq")
                nc.scalar.activation(sc2[:], xr[:, rt], AF.Square, accum_out=ssq[:])
                m = scr.tile([P, 1], F32, tag="m")
                nc.vector.tensor_scalar_mul(m[:], ssum[:], 1.0 / D)
                var = scr.tile([P, 1], F32, tag="var")
                nc.vector.tensor_scalar_mul(var[:], ssq[:], 1.0 / D)
                m2 = scr.tile([P, 1], F32, tag="m2")
                nc.vector.tensor_mul(m2[:], m[:], m[:])
                nc.vector.tensor_sub(var[:], var[:], m2[:])
                nc.vector.tensor_scalar_add(var[:], var[:], EPS)
                std = scr.tile([P, 1], F32, tag="std")
                nc.scalar.activation(std[:], var[:], AF.Sqrt)
                rinv = scr.tile([P, 1], F32, tag="rinv")
                nc.vector.reciprocal(rinv[:], std[:])
                nc.vector.tensor_scalar(
                    xn[:, rt], xr[:, rt], m[:], rinv[:], OP.subtract, OP.mult
                )

            # Xn^T via PE transpose
            pa2_cm = tc.tile_pool(name="pa2", bufs=1)
            pa2 = pa2_cm.__enter__()
            ptp_cm = tc.tile_pool(name="ptp", bufs=2, space="PSUM")
            ptp = ptp_cm.__enter__()
            xnt = pa2.tile([P, DT, R], BF16, tag="xnt")
            wq_sb = pa2.tile([P, DT, D], BF16, tag="wq")
            nc.sync.dma_start(
                wq_sb[:],
                wall[OFF_WQ:OFF_WK].rearrange("(kt p m) -> p kt m", p=P, m=D),
            )
            for rt in range(RT):
                for cb in range(DT):
                    tp = ptp.tile([P, P], BF16, tag="tp")
                    nc.tensor.transpose(tp[:], xn[:, rt, cb * P:(cb + 1) * P], ident[:])
                    nc.vector.tensor_copy(xnt[:, cb, rt * P:(rt + 1) * P], tp[:])

            # Q^T Dout-major, M=128 matmuls straight into qt128
            for mt in range(DT):
                for qc in range(QC):
                    pq = ps.tile([P, 512], F32, tag="mm")
                    for kt in range(DT):
                        nc.tensor.matmul(
                            pq[:],
                            wq_sb[:, kt, mt * P:(mt + 1) * P],
                            xnt[:, kt, qc * 512:(qc + 1) * 512],
                            start=(kt == 0), stop=(kt == DT - 1),
                        )
                    nc.scalar.activation(
                        qt128[:, mt, qc * 512:(qc + 1) * 512], pq[:], AF.Identity,
                        bias=bq_sb[:, mt:mt + 1],
                    )
            ptp_cm.__exit__(None, None, None)
            pa2_cm.__exit__(None, None, None)  # free xnt, wq
            pa1_cm.__exit__(None, None, None)  # free xr

            # K^T head-major and V row-major
            pa3_cm = tc.tile_pool(name="pa3", bufs=1)
            pa3 = pa3_cm.__enter__()
            wk_sb = pa3.tile([P, DT, D], BF16, tag="wk")
            nc.sync.dma_start(
                wk_sb[:],
                wall[OFF_WK:OFF_WV].rearrange("(kt p m) -> p kt m", p=P, m=D),
            )
            wv_sb = pa3.tile([P, DT, D], BF16, tag="wv")
            nc.sync.dma_start(
                wv_sb[:],
                wall[OFF_WV:OFF_WMH].rearrange("(kt p m) -> p kt m", p=P, m=D),
            )
            nc.vector.memset(vaug[:, :, :, DH:DH + 1], 1.0)

            for khalf in range(2):
                yt_sb = pa3.tile([P, DT, N // 2], BF16, tag="yt", bufs=1)
                nc.sync.dma_start(
                    yt_sb[:],
                    ytall[:].rearrange("(kt p n) -> p kt n", p=P, n=N)
                    [:, :, khalf * (N // 2):(khalf + 1) * (N // 2)],
                )
                for mt in range(DT):
                    for kcl in range(KC // 2):
                        kc = khalf * (KC // 2) + kcl
                        pk = ps.tile([P, 512], F32, tag="mm")
                        for kt in range(DT):
                            nc.tensor.matmul(
                                pk[:],
                                wk_sb[:, kt, mt * P:(mt + 1) * P],
                                yt_sb[:, kt, kcl * 512:(kcl + 1) * 512],
                                start=(kt == 0), stop=(kt == DT - 1),
                            )
                        nc.scalar.copy(kt2[:, mt, kc * 512:(kc + 1) * 512], pk[:])
                for rtl in range(KT16 // 2):
                    rt = khalf * (KT16 // 2) + rtl
                    pv = ps.tile([P, 512], F32, tag="mm")
                    for kt in range(DT):
                        nc.tensor.matmul(
                            pv[:],
                            yt_sb[:, kt, rtl * P:(rtl + 1) * P],
                            wv_sb[:, kt, :],
                            start=(kt == 0), stop=(kt == DT - 1),
                        )
                    nc.scalar.copy(
                        vaug[:, rt, :, 0:DH], pv[:].rearrange("p (h d) -> p h d", h=H)
                    )
            pa3_cm.__exit__(None, None, None)  # free yt, wk, wv

            # ================= phase B: attention =================
            pb_cm = tc.tile_pool(name="pb", bufs=1)
            pb = pb_cm.__enter__()
            pbig_cm = tc.tile_pool(name="pbig", bufs=1, space="PSUM")
            pbig = pbig_cm.__enter__()
            for hp in range(H // 2):
                ats = [pb.tile([P, KT16, R], BF16, tag="at0", bufs=1, name="at0"),
                       pb.tile([P, KT16, R], BF16, tag="at1", bufs=1, name="at1")]
                for kt in range(KT16):
                    pse = pbig.tile([P, R], F32, tag="bigE")
                    pso = pbig.tile([P, R], F32, tag="bigO")
                    for qc in range(QC):
                        nc.tensor.matmul(
                            pse[:, qc * 512:(qc + 1) * 512],
                            kt2[0:DH, hp, kt * P:(kt + 1) * P],
                            qt128[0:DH, hp, qc * 512:(qc + 1) * 512],
                            start=True, stop=True,
                        )
                        nc.tensor.matmul(
                            pso[:, qc * 512:(qc + 1) * 512],
                            kt2[DH:P, hp, kt * P:(kt + 1) * P],
                            qt128[DH:P, hp, qc * 512:(qc + 1) * 512],
                            start=True, stop=True, tile_position=(DH, 0),
                        )
                    nc.scalar.activation(ats[0][:, kt, :], pse[:], AF.Exp, scale=SCALE)
                    nc.scalar.activation(ats[1][:, kt, :], pso[:], AF.Exp, scale=SCALE)
                for par in range(2):
                    h = 2 * hp + par
                    at = ats[par]
                    for qc in range(QC):
                        pav = ps.tile([P, 512], F32, tag="mm")
                        for kt in range(KT16):
                            nc.tensor.matmul(
                                pav[0:DH + 1, :],
                                vaug[:, kt, h, :],
                                at[:, kt, qc * 512:(qc + 1) * 512],
                                start=(kt == 0), stop=(kt == KT16 - 1),
                            )
                        ot_sb = scr.tile([DH, 512], F32, tag="otsb", bufs=2)
                        nc.vector.tensor_copy(ot_sb[:], pav[0:DH, :])
                        rd_sb = scr.tile([DH + 1, 512], F32, tag="rds", bufs=2)
                        nc.vector.reciprocal(rd_sb[DH:DH + 1, :], pav[DH:DH + 1, :])
                        rd_sbr = scr.tile([DH + 1, 512], F32R, tag="rdsr", bufs=2)
                        nc.vector.tensor_copy(rd_sbr[DH:DH + 1, :], rd_sb[DH:DH + 1, :])
                        pbc = ps.tile([DH, 512], F32, tag="mm")
                        nc.tensor.matmul(
                            pbc[:], ones2d[DH:DH + 1, :], rd_sbr[DH:DH + 1, :],
                            start=True, stop=True,
                        )
                        nc.vector.tensor_mul(
                            mt_sb[:, h, qc * 512:(qc + 1) * 512], ot_sb[:], pbc[:]
                        )
            pbig_cm.__exit__(None, None, None)
            pb_cm.__exit__(None, None, None)  # free at

            # ================= phase C: mix + residual =================
            for mt in range(DT):
                for qc in range(QC):
                    pm = ps.tile([P, 512], F32, tag="mm")
                    for h in range(H):
                        nc.tensor.matmul(
                            pm[:],
                            wmh_sb[:, h, mt * P:(mt + 1) * P],
                            mt_sb[:, h, qc * 512:(qc + 1) * 512],
                            start=(h == 0), stop=(h == H - 1),
                        )
                    q = qc * 512
                    nc.vector.tensor_add(
                        hxt[:, mt, q:q + 512], pm[:], qt128[:, mt, q:q + 512]
                    )
                    nc.vector.tensor_scalar_add(
                        hxt[:, mt, q:q + 512], hxt[:, mt, q:q + 512], bm_sb[:, mt:mt + 1]
                    )
            pattn_cm.__exit__(None, None, None)  # free qt128/kt2/vaug/mt/wmh

            # ================= phase D: LN1 (feature-major) + FFN =================
            pd_cm = tc.tile_pool(name="pd", bufs=1)
            pd = pd_cm.__enter__()
            pst_cm = tc.tile_pool(name="pst", bufs=2, space="PSUM")
            pst = pst_cm.__enter__()
            w1_sb = pd.tile([P, DT, DFF], BF16, tag="w1")
            nc.gpsimd.dma_start(
                w1_sb[:],
                wall[OFF_W1:OFF_W2].rearrange("(kt p m) -> p kt m", p=P, m=DFF),
            )
            w2_sb = pd.tile([P, FT, D], BF16, tag="w2")
            nc.gpsimd.dma_start(
                w2_sb[:],
                wall[OFF_W2:WTOT].rearrange("(kt p m) -> p kt m", p=P, m=D),
            )

            hxn = pd.tile([P, DT, R], BF16, tag="hxn")
            for qc in range(QC):
                q = qc * 512
                ps_s = pst.tile([1, 512], F32, tag="st")
                for dt in range(DT):
                    nc.tensor.matmul(
                        ps_s[:], onescol[:], hxt[:, dt, q:q + 512],
                        start=(dt == 0), stop=(dt == DT - 1),
                    )
                mean = scr.tile([1, 512], F32, tag="mean", bufs=1)
                nc.vector.tensor_scalar_mul(mean[:], ps_s[:], 1.0 / D)
                ps_q = pst.tile([1, 512], F32, tag="st")
                for dt in range(DT):
                    sqs = scr.tile([P, 512], F32R, tag="sqs", bufs=2)
                    nc.vector.tensor_mul(sqs[:], hxt[:, dt, q:q + 512], hxt[:, dt, q:q + 512])
                    nc.tensor.matmul(
                        ps_q[:], onescol[:], sqs[:],
                        start=(dt == 0), stop=(dt == DT - 1),
                    )
                var = scr.tile([1, 512], F32, tag="lvar", bufs=1)
                nc.vector.tensor_scalar_mul(var[:], ps_q[:], 1.0 / D)
                m2 = scr.tile([1, 512], F32, tag="lm2", bufs=1)
                nc.vector.tensor_mul(m2[:], mean[:], mean[:])
                nc.vector.tensor_sub(var[:], var[:], m2[:])
                nc.vector.tensor_scalar_add(var[:], var[:], EPS)
                std = scr.tile([1, 512], F32, tag="lstd", bufs=1)
                nc.scalar.activation(std[:], var[:], AF.Sqrt)
                rstd32 = scr.tile([1, 512], F32, tag="lrstd32", bufs=1)
                nc.vector.reciprocal(rstd32[:], std[:])
                rstd = scr.tile([1, 512], F32R, tag="lrstd", bufs=1)
                nc.vector.tensor_copy(rstd[:], rstd32[:])
                mrs = scr.tile([1, 512], F32R, tag="lmrs", bufs=1)
                nc.vector.tensor_mul(mrs[:], mean[:], rstd32[:])
                pb_r = ps.tile([P, 512], F32, tag="mm")
                nc.tensor.matmul(pb_r[:], ones1x128[:], rstd[:], start=True, stop=True)
                pb_m = ps.tile([P, 512], F32, tag="mm")
                nc.tensor.matmul(pb_m[:], ones1x128[:], mrs[:], start=True, stop=True)
                for dt in range(DT):
                    nc.vector.tensor_mul(hxn[:, dt, q:q + 512], hxt[:, dt, q:q + 512], pb_r[:])
                    nc.vector.tensor_sub(hxn[:, dt, q:q + 512], hxn[:, dt, q:q + 512], pb_m[:])

            gt = pd.tile([P, FT, R], BF16, tag="gt")
            for ft in range(FT):
                for qc in range(QC):
                    pf = ps.tile([P, 512], F32, tag="mm")
                    for kt in range(DT):
                        nc.tensor.matmul(
                            pf[:],
                            w1_sb[:, kt, ft * P:(ft + 1) * P],
                            hxn[:, kt, qc * 512:(qc + 1) * 512],
                            start=(kt == 0), stop=(kt == DT - 1),
                        )
                    if not gelu_tanh:
                        nc.scalar.activation(
                            gt[:, ft, qc * 512:(qc + 1) * 512], pf[:], AF.Gelu,
                            bias=bb1_sb[:, ft:ft + 1],
                        )
                    else:
                        # sim-only tanh-approx gelu (AF.Gelu unimplemented there)
                        ub = scr.tile([P, 512], F32, tag="gub", bufs=2)
                        nc.scalar.activation(ub[:], pf[:], AF.Identity,
                                             bias=bb1_sb[:, ft:ft + 1])
                        u2 = scr.tile([P, 512], F32, tag="gu2", bufs=2)
                        nc.vector.tensor_mul(u2[:], ub[:], ub[:])
                        nc.vector.tensor_scalar_mul(u2[:], u2[:], 0.044715)
                        nc.vector.tensor_scalar_add(u2[:], u2[:], 1.0)
                        nc.vector.tensor_mul(u2[:], u2[:], ub[:])
                        nc.vector.tensor_scalar_mul(u2[:], u2[:], 0.7978845608028654)
                        th = scr.tile([P, 512], F32, tag="gth", bufs=2)
                        nc.scalar.activation(th[:], u2[:], AF.Tanh)
                        nc.vector.tensor_scalar_add(th[:], th[:], 1.0)
                        nc.vector.tensor_mul(th[:], th[:], ub[:])
                        nc.vector.tensor_scalar_mul(
                            gt[:, ft, qc * 512:(qc + 1) * 512], th[:], 0.5)

            out_sb = pd.tile([P, DT, R], BF16, tag="outsb")
            for mt in range(DT):
                for qc in range(QC):
                    po = ps.tile([P, 512], F32, tag="mm")
                    for kt in range(FT):
                        nc.tensor.matmul(
                            po[:],
                            w2_sb[:, kt, mt * P:(mt + 1) * P],
                            gt[:, kt, qc * 512:(qc + 1) * 512],
                            start=(kt == 0), stop=(kt == FT - 1),
                        )
                    q = qc * 512
                    nc.vector.tensor_add(
                        out_sb[:, mt, q:q + 512], po[:], hxt[:, mt, q:q + 512]
                    )
                    nc.vector.tensor_scalar_add(
                        out_sb[:, mt, q:q + 512], out_sb[:, mt, q:q + 512],
                        bb2_sb[:, mt:mt + 1],
                    )
            nc.gpsimd.dma_start(o_d.ap().rearrange("(mt p) n -> p mt n", p=P), out_sb[:])
            pst_cm.__exit__(None, None, None)
            pd_cm.__exit__(None, None, None)

    nc.compile()
    return nc


def _make_runner(nc, n_cores):
    """Build the reusable jitted SPMD executor for `nc`.

    Mirrors concourse.bass2jax.run_bass_via_pjrt's multi-core branch but
    constructs the jit closure ONCE so repeat calls hit the jit cache
    (run_bass_via_pjrt builds a fresh closure per call, forcing a full
    retrace + executable re-import through the axon tunnel every call).
    """
    import jax
    from jax.sharding import Mesh, PartitionSpec
    from jax.experimental.shard_map import shard_map
    from concourse import bass2jax

    if jax.default_backend() == "axon":
        bass2jax.install_neuronx_cc_hook()

    partition_name = nc.partition_id_tensor.name if nc.partition_id_tensor else None
    in_names, out_names, out_avals, zero_outs = [], [], [], []
    for alloc in nc.m.functions[0].allocations:
        if not isinstance(alloc, mybir.MemoryLocationSet):
            continue
        name = alloc.memorylocations[0].name
        if alloc.kind == "ExternalInput":
            if name != partition_name:
                in_names.append(name)
        elif alloc.kind == "ExternalOutput":
            shape = tuple(alloc.tensor_shape)
            dtype = mybir.dt.np(alloc.dtype)
            out_names.append(name)
            out_avals.append(jax.core.ShapedArray(shape, dtype))
            zero_outs.append(np.zeros((n_cores * shape[0], *shape[1:]), dtype))
    n_params = len(in_names)
    n_outs = len(out_names)
    bind_names = tuple(in_names + out_names + ([partition_name] if partition_name else []))
    donate = tuple(range(n_params, n_params + n_outs))

    def _body(*args):
        operands = list(args)
        if partition_name is not None:
            operands.append(bass2jax.partition_id_tensor())
        outs = bass2jax._bass_exec_p.bind(
            *operands,
            out_avals=tuple(out_avals),
            in_names=bind_names,
            out_names=tuple(out_names),
            lowering_input_output_aliases=(),
            sim_require_finite=True,
            sim_require_nnan=True,
            nc=nc,
        )
        return tuple(outs)

    devices = jax.devices()[:n_cores]
    assert len(devices) == n_cores, f"need {n_cores} devices, have {len(jax.devices())}"
    mesh = Mesh(np.asarray(devices), ("core",))
    in_specs = (PartitionSpec("core"),) * (n_params + n_outs)
    out_specs = (PartitionSpec("core"),) * n_outs
    # cpu (sim) doesn't implement buffer donation; the unaliased
    # jax.buffer_donor annotation trips the sim lowering's check.
    jit_kwargs = {} if jax.default_backend() == "cpu" else dict(donate_argnums=donate)
    fn = jax.jit(
        shard_map(_body, mesh=mesh, in_specs=in_specs, out_specs=out_specs,
                  check_rep=False),
        keep_unused=True, **jit_kwargs,
    )
    sharding = jax.sharding.NamedSharding(mesh, PartitionSpec("core"))
    import jax.numpy as jnp

    zero_shapes = [z.shape for z in zero_outs]
    zero_dtypes = [z.dtype for z in zero_outs]
    zeros_fn = jax.jit(
        lambda: tuple(jnp.zeros(s, d) for s, d in zip(zero_shapes, zero_dtypes)),
        out_shardings=tuple(sharding for _ in zero_outs),
    )
    return dict(fn=fn, in_names=in_names, out_names=out_names,
                out_avals=out_avals, zero_outs=zero_outs, n_cores=n_cores,
                sharding=sharding, zeros_fn=zeros_fn)


def _run_cached(runner, global_in):
    """Run the cached executor on pre-built GLOBAL (concat-over-cores)
    arrays. Inputs are device_put asynchronously in order (host work and
    the serialized tunnel uploads overlap); the donated zero output
    buffers are created on-device (no wire bytes)."""
    import time
    import jax

    n_cores = runner["n_cores"]
    # The terminal-side worker takes minutes to restart after a previous
    # process's teardown; fresh connections see transient
    # NRT_EXEC_UNIT_UNRECOVERABLE / UNAVAILABLE until it's back. Retry
    # patiently — this only ever triggers on the first call of a process.
    for attempt in range(30):
        try:
            dev_in = [jax.device_put(global_in[name], runner["sharding"])
                      for name in runner["in_names"]]
            zeros = runner["zeros_fn"]()
            out_arrs = runner["fn"](*dev_in, *zeros)
            np_out = [np.asarray(a) for a in out_arrs]
            break
        except Exception as e:  # noqa: BLE001
            msg = f"{type(e).__name__}: {e}"
            transient = ("UNRECOVERABLE" in msg or "UNAVAILABLE" in msg
                         or "NRT_" in msg or "PassThrough" in msg)
            if not transient or attempt == 29:
                raise
            time.sleep(40)
    return {
        name: np_out[i].reshape(n_cores, *runner["out_avals"][i].shape)
        for i, name in enumerate(runner["out_names"])
    }


def kernel(X, Y, Wq, bq, Wk, bk, Wv, bv, Wm, bm, g0, b0, g1, b1, W1, bb1, W2, bb2,
           **_ignored):
    X = np.asarray(X, dtype=np.float32)
    Y = np.asarray(Y, dtype=np.float32)
    f32 = lambda a: np.asarray(a, dtype=np.float32)
    Wq, bq, Wk, Wv, bv, Wm, bm = map(f32, (Wq, bq, Wk, Wv, bv, Wm, bm))
    g0, b0, g1, b1, W1, bb1, W2, bb2 = map(f32, (g0, b0, g1, b1, W1, bb1, W2, bb2))

    # host-side exact folds (f32), then quantize the wire copies to bf16.
    # Weights are packed into one flat blob: core c uploads
    # wblob[c*WSH:(c+1)*WSH] and the device AllGathers the full blob —
    # the global (concat-over-cores) wsh array IS wblob, no copies needed.
    # Likewise global x = bf16(X) reshaped, global ysh = bf16(Y^T) raveled.
    bqv = b0 @ Wq + bq
    bmv = bv @ Wm + bm
    bb1v = b1 @ W1 + bb1
    wblob = np.empty(WTOT, dtype=NPBF16)
    wblob[OFF_WQ:OFF_WK] = (g0[:, None] * Wq).astype(NPBF16).ravel()
    wblob[OFF_WK:OFF_WV] = Wk.astype(NPBF16).ravel()
    wblob[OFF_WV:OFF_WMH] = Wv.astype(NPBF16).ravel()
    wblob[OFF_WMH:OFF_W1] = np.ascontiguousarray(
        Wm.reshape(H, DH, D).transpose(1, 0, 2)).astype(NPBF16).ravel()
    wblob[OFF_W1:OFF_W2] = (g1[:, None] * W1).astype(NPBF16).ravel()
    wblob[OFF_W2:WTOT] = W2.astype(NPBF16).ravel()
    idm = np.eye(P, dtype=NPBF16)
    on1 = np.ones((P, 1), dtype=np.float32)
    on2 = np.ones((1, P), dtype=np.float32)
    onp = np.ones((DH + 1, DH), dtype=np.float32)
    tile8 = lambda a: np.tile(a, (NCORES,) + (1,) * (a.ndim - 1))
    global_in = dict(
        x=X.astype(NPBF16).reshape(NCORES * R, D),
        ysh=Y.transpose(0, 2, 1).astype(NPBF16).reshape(NCORES * YSH),
        wsh=wblob,
        bq=tile8(bqv), bm=tile8(bmv), bb1=tile8(bb1v), bb2=tile8(bb2),
        idm=tile8(idm), on1=tile8(on1), on2=tile8(on2), onp=tile8(onp),
    )

    if "nc" not in _cache:
        _cache["nc"] = _build(gelu_tanh=_cache.get("gelu_tanh", False))
    nc = _cache["nc"]

    if _cache.get("run_kwargs"):
        in_maps = []
        for c in range(NCORES):
            b, half = c // 2, c % 2
            in_maps.append(dict(
                x=np.ascontiguousarray(global_in["x"][c * R:(c + 1) * R]),
                ysh=np.ascontiguousarray(global_in["ysh"][c * YSH:(c + 1) * YSH]),
                wsh=np.ascontiguousarray(wblob[c * WSH:(c + 1) * WSH]),
                bq=bqv, bm=bmv, bb1=bb1v, bb2=bb2, idm=idm,
                on1=on1, on2=on2, onp=onp,
            ))
        res = run_bass_kernel_spmd(nc, in_maps, core_ids=list(range(NCORES)),
                                   **_cache["run_kwargs"])
        _cache["last"] = res
        o_g = np.stack([res.results[c]["o"] for c in range(NCORES)])
    else:
        if "runner" not in _cache:
            _cache["runner"] = _make_runner(nc, NCORES)
        o_g = _run_cached(_cache["runner"], global_in)["o"]
        _cache["last"] = None
    # o_g: [8, D, R] bf16, core c = (batch c//2, row-half c%2)
    out = o_g.transpose(0, 2, 1).astype(np.float32).reshape(B, N, D)
    return out


# revision 26
# speedup vs baseline: 2.5714x; 1.0384x over previous
"""Trainium2 Bass kernel for a pre-LN transformer block (attention + FFN).

Sharding: 8 cores = (batch b = c//2) x (query-row half = c%2). Each core
computes 1024 query rows end-to-end; K/V for its batch are computed on-core
(duplicated across the 2 cores sharing a batch). No collectives.

Math folds done on host (exact, in f32):
  - LN gains/biases folded into Wq/W1 (gamma row-scales W, beta@W folds into bias)
  - bk dropped (softmax row-shift invariant), bv folded into mix bias
Device computes plain (x-mean)*rstd for both LNs.

Wire format is bf16 for all large tensors (activations, weights, output);
biases and LN scratch stay f32, the residual stream stays f32 on device.
The axon tunnel (~33MB/s) dominates end-to-end latency, so halving bytes
halves latency; bf16 error is ~1e-3 vs the 2e-2 gate.

The PJRT executable is built and jit-compiled ONCE (module-level cache);
per-call work is host prep + transfer + execute. Routing every call
through bass_utils.run_bass_kernel_spmd would rebuild the jit closure
each time (full retrace + NEFF re-import over the tunnel, ~20s/call);
the cached runner binds the same _bass_exec_p primitive that
run_bass_kernel_spmd's axon path (bass2jax.run_bass_via_pjrt) uses, so
the on-device execution is identical. Set _cache["run_kwargs"]
(e.g. trace=True) to route through run_bass_kernel_spmd instead.
"""

import sys

sys.path.insert(0, "/opt/trn_rl_repo")

import numpy as np
import ml_dtypes

import concourse.bass as bass
import concourse.bacc as bacc
import concourse.mybir as mybir
import concourse.tile as tile
from concourse.bass_utils import run_bass_kernel_spmd

F32 = mybir.dt.float32
F32R = mybir.dt.float32r
BF16 = mybir.dt.bfloat16
FP8 = mybir.dt.float8e3          # e3m4: 4 mantissa bits, max 15.5
AF = mybir.ActivationFunctionType
OP = mybir.AluOpType
NPBF16 = ml_dtypes.bfloat16
NPFP8 = ml_dtypes.float8_e3m4

B, N, D, H = 4, 2048, 512, 8
DH = D // H            # 64
DFF = 4 * D            # 2048
R = 1024               # query rows per core
P = 128
EPS = 1e-5
SCALE = 1.0 / float(np.sqrt(D))

DT = D // P            # 4  Din 128-tiles
RT = R // P            # 8  query-row 128-tiles of this core
KT16 = N // P          # 16 key 128-tiles
QC = R // 512          # 2  query 512-chunks
KC = N // 512          # 4  key 512-chunks
FT = DFF // P          # 16 dff 128-tiles

NCORES = 8

# packed bf16 weight blob (flat element offsets); each core uploads 1/8 and
# the full blob is AllGather'd on-device over NeuronLink — the axon tunnel
# (~30MB/s) is the bottleneck, so shipping weights once instead of 8x wins.
OFF_WQ = 0
OFF_WK = OFF_WQ + D * D
OFF_WV = OFF_WK + D * D
OFF_WMH = OFF_WV + D * D
OFF_W1 = OFF_WMH + DH * H * D
OFF_W2 = OFF_W1 + D * DFF
WTOT = OFF_W2 + DFF * D          # 3145728
WSH = WTOT // NCORES             # 393216
YTOT = D * N                     # yt blob per batch (1048576)
YSH = YTOT // 2                  # each core of a batch pair uploads half

_cache = {}


def _build(gelu_tanh=False):
    nc = bacc.Bacc("TRN2", target_bir_lowering=False, debug=False, num_devices=8)
    dt_ = nc.dram_tensor
    x_d = dt_("x", [R, D], BF16, kind="ExternalInput")
    # Y only feeds K/V; fp8 quantization noise washes out through the
    # softmax average over 2048 keys (~1e-3 absolute on the output), so
    # ship it in e3m4 and dequantize to bf16 on device.
    ysh_d = dt_("ysh", [YSH], FP8, kind="ExternalInput")
    wsh_d = dt_("wsh", [WSH], BF16, kind="ExternalInput")
    bq_d = dt_("bq", [D], F32, kind="ExternalInput")
    bm_d = dt_("bm", [D], F32, kind="ExternalInput")
    bb1_d = dt_("bb1", [DFF], F32, kind="ExternalInput")
    bb2_d = dt_("bb2", [D], F32, kind="ExternalInput")
    idm_d = dt_("idm", [P, P], BF16, kind="ExternalInput")
    on1_d = dt_("on1", [P, 1], F32R, kind="ExternalInput")
    on2_d = dt_("on2", [1, P], F32R, kind="ExternalInput")
    onp_d = dt_("onp", [DH + 1, DH], F32R, kind="ExternalInput")
    o_d = dt_("o", [D, R], BF16, kind="ExternalOutput")

    with tile.TileContext(nc) as tc:
        with (
            tc.tile_pool(name="sb", bufs=1) as sb,
            tc.tile_pool(name="scr", bufs=2) as scr,
            tc.tile_pool(name="ps", bufs=4, space="PSUM") as ps,
            tc.tile_pool(name="dram", bufs=1, space="DRAM") as dram,
        ):
            # ---- gather the sharded uploads (bounce via DRAM: collectives
            # can't read I/O tensors directly) ----
            win = dram.tile([WSH], BF16, tag="win")
            nc.gpsimd.dma_start(win[:], wsh_d.ap())
            wall = dram.tile([WTOT], BF16, tag="wall")
            nc.gpsimd.collective_compute(
                "AllGather", OP.bypass,
                replica_groups=[list(range(NCORES))],
                ins=[win.opt()], outs=[wall.opt()],
            )
            yin = dram.tile([YSH], FP8, tag="yin")
            nc.gpsimd.dma_start(yin[:], ysh_d.ap())
            ytall = dram.tile([YTOT], FP8, tag="ytall")
            nc.gpsimd.collective_compute(
                "AllGather", OP.bypass,
                replica_groups=[[2 * b, 2 * b + 1] for b in range(B)],
                ins=[yin.opt()], outs=[ytall.opt()],
            )

            # ---- constants / biases (persist) ----
            ident = sb.tile([P, P], BF16, tag="ident")
            nc.sync.dma_start(ident[:], idm_d.ap())
            ones1x128 = sb.tile([1, P], F32R, tag="o1x128")
            nc.sync.dma_start(ones1x128[:], on2_d.ap())
            onescol = sb.tile([P, 1], F32R, tag="ocol")
            nc.sync.dma_start(onescol[:], on1_d.ap())
            ones2d = sb.tile([DH + 1, DH], F32R, tag="onp")
            nc.sync.dma_start(ones2d[:], onp_d.ap())
            bq_sb = sb.tile([P, DT], F32, tag="bq")
            nc.sync.dma_start(bq_sb[:], bq_d.ap().rearrange("(mt p) -> p mt", p=P))
            bm_sb = sb.tile([P, DT], F32, tag="bm")
            nc.sync.dma_start(bm_sb[:], bm_d.ap().rearrange("(mt p) -> p mt", p=P))
            bb1_sb = sb.tile([P, FT], F32, tag="bb1")
            nc.sync.dma_start(bb1_sb[:], bb1_d.ap().rearrange("(ft p) -> p ft", p=P))
            bb2_sb = sb.tile([P, DT], F32, tag="bb2")
            nc.sync.dma_start(bb2_sb[:], bb2_d.ap().rearrange("(mt p) -> p mt", p=P))
            # residual stream lives whole kernel
            hxt = sb.tile([P, DT, R], F32R, tag="hxt")

            # attention-lifetime pool: closed after mix
            pattn_cm = tc.tile_pool(name="pattn", bufs=1)
            pattn = pattn_cm.__enter__()
            qt128 = pattn.tile([P, DT, R], BF16, tag="qt128")
            kt2 = pattn.tile([P, DT, N], BF16, tag="kt2")
            vaug = pattn.tile([P, KT16, H, DH + 1], BF16, tag="vaug")
            mt_sb = pattn.tile([DH, H, R], BF16, tag="mt")
            wmh_sb = pattn.tile([DH, H, D], BF16, tag="wmh")
            nc.gpsimd.dma_start(
                wmh_sb[:],
                wall[OFF_WMH:OFF_W1].rearrange("(d h m) -> d h m", h=H, m=D),
            )

            # ================= phase A: LN0, transposes, Q/K/V =================
            pa1_cm = tc.tile_pool(name="pa1", bufs=1)
            pa1 = pa1_cm.__enter__()
            xr = pa1.tile([P, RT, D], BF16, tag="xr")
            nc.sync.dma_start(xr[:], x_d.ap().rearrange("(rt p) d -> p rt d", p=P))
            xn = xr
            for rt in range(RT):
                sc1 = scr.tile([P, D], F32, tag="lnscr")
                ssum = scr.tile([P, 1], F32, tag="ssum")
                nc.scalar.activation(sc1[:], xr[:, rt], AF.Identity, accum_out=ssum[:])
                sc2 = scr.tile([P, D], F32, tag="lnscr")
                ssq = scr.tile([P, 1], F32, tag="ssq")
                nc.scalar.activation(sc2[:], xr[:, rt], AF.Square, accum_out=ssq[:])
                m = scr.tile([P, 1], F32, tag="m")
                nc.vector.tensor_scalar_mul(m[:], ssum[:], 1.0 / D)
                var = scr.tile([P, 1], F32, tag="var")
                nc.vector.tensor_scalar_mul(var[:], ssq[:], 1.0 / D)
                m2 = scr.tile([P, 1], F32, tag="m2")
                nc.vector.tensor_mul(m2[:], m[:], m[:])
                nc.vector.tensor_sub(var[:], var[:], m2[:])
                nc.vector.tensor_scalar_add(var[:], var[:], EPS)
                std = scr.tile([P, 1], F32, tag="std")
                nc.scalar.activation(std[:], var[:], AF.Sqrt)
                rinv = scr.tile([P, 1], F32, tag="rinv")
                nc.vector.reciprocal(rinv[:], std[:])
                nc.vector.tensor_scalar(
                    xn[:, rt], xr[:, rt], m[:], rinv[:], OP.subtract, OP.mult
                )

            # Xn^T via PE transpose
            pa2_cm = tc.tile_pool(name="pa2", bufs=1)
            pa2 = pa2_cm.__enter__()
            ptp_cm = tc.tile_pool(name="ptp", bufs=2, space="PSUM")
            ptp = ptp_cm.__enter__()
            xnt = pa2.tile([P, DT, R], BF16, tag="xnt")
            wq_sb = pa2.tile([P, DT, D], BF16, tag="wq")
            nc.sync.dma_start(
                wq_sb[:],
                wall[OFF_WQ:OFF_WK].rearrange("(kt p m) -> p kt m", p=P, m=D),
            )
            for rt in range(RT):
                for cb in range(DT):
                    tp = ptp.tile([P, P], BF16, tag="tp")
                    nc.tensor.transpose(tp[:], xn[:, rt, cb * P:(cb + 1) * P], ident[:])
                    nc.vector.tensor_copy(xnt[:, cb, rt * P:(rt + 1) * P], tp[:])

            # Q^T Dout-major, M=128 matmuls straight into qt128
            for mt in range(DT):
                for qc in range(QC):
                    pq = ps.tile([P, 512], F32, tag="mm")
                    for kt in range(DT):
                        nc.tensor.matmul(
                            pq[:],
                            wq_sb[:, kt, mt * P:(mt + 1) * P],
                            xnt[:, kt, qc * 512:(qc + 1) * 512],
                            start=(kt == 0), stop=(kt == DT - 1),
                        )
                    nc.scalar.activation(
                        qt128[:, mt, qc * 512:(qc + 1) * 512], pq[:], AF.Identity,
                        bias=bq_sb[:, mt:mt + 1],
                    )
            ptp_cm.__exit__(None, None, None)
            pa2_cm.__exit__(None, None, None)  # free xnt, wq
            pa1_cm.__exit__(None, None, None)  # free xr

            # K^T head-major and V row-major
            pa3_cm = tc.tile_pool(name="pa3", bufs=1)
            pa3 = pa3_cm.__enter__()
            wk_sb = pa3.tile([P, DT, D], BF16, tag="wk")
            nc.sync.dma_start(
                wk_sb[:],
                wall[OFF_WK:OFF_WV].rearrange("(kt p m) -> p kt m", p=P, m=D),
            )
            wv_sb = pa3.tile([P, DT, D], BF16, tag="wv")
            nc.sync.dma_start(
                wv_sb[:],
                wall[OFF_WV:OFF_WMH].rearrange("(kt p m) -> p kt m", p=P, m=D),
            )
            nc.vector.memset(vaug[:, :, :, DH:DH + 1], 1.0)

            for khalf in range(2):
                yt8 = pa3.tile([P, DT, N // 2], FP8, tag="yt8", bufs=1)
                nc.sync.dma_start(
                    yt8[:],
                    ytall[:].rearrange("(kt p n) -> p kt n", p=P, n=N)
                    [:, :, khalf * (N // 2):(khalf + 1) * (N // 2)],
                )
                yt_sb = pa3.tile([P, DT, N // 2], BF16, tag="yt", bufs=1)
                nc.scalar.copy(yt_sb[:], yt8[:])
                for mt in range(DT):
                    for kcl in range(KC // 2):
                        kc = khalf * (KC // 2) + kcl
                        pk = ps.tile([P, 512], F32, tag="mm")
                        for kt in range(DT):
                            nc.tensor.matmul(
                                pk[:],
                                wk_sb[:, kt, mt * P:(mt + 1) * P],
                                yt_sb[:, kt, kcl * 512:(kcl + 1) * 512],
                                start=(kt == 0), stop=(kt == DT - 1),
                            )
                        nc.scalar.copy(kt2[:, mt, kc * 512:(kc + 1) * 512], pk[:])
                for rtl in range(KT16 // 2):
                    rt = khalf * (KT16 // 2) + rtl
                    pv = ps.tile([P, 512], F32, tag="mm")
                    for kt in range(DT):
                        nc.tensor.matmul(
                            pv[:],
                            yt_sb[:, kt, rtl * P:(rtl + 1) * P],
                            wv_sb[:, kt, :],
                            start=(kt == 0), stop=(kt == DT - 1),
                        )
                    nc.scalar.copy(
                        vaug[:, rt, :, 0:DH], pv[:].rearrange("p (h d) -> p h d", h=H)
                    )
            pa3_cm.__exit__(None, None, None)  # free yt, wk, wv

            # ================= phase B: attention =================
            pb_cm = tc.tile_pool(name="pb", bufs=1)
            pb = pb_cm.__enter__()
            pbig_cm = tc.tile_pool(name="pbig", bufs=1, space="PSUM")
            pbig = pbig_cm.__enter__()
            for hp in range(H // 2):
                ats = [pb.tile([P, KT16, R], BF16, tag="at0", bufs=1, name="at0"),
                       pb.tile([P, KT16, R], BF16, tag="at1", bufs=1, name="at1")]
                for kt in range(KT16):
                    pse = pbig.tile([P, R], F32, tag="bigE")
                    pso = pbig.tile([P, R], F32, tag="bigO")
                    for qc in range(QC):
                        nc.tensor.matmul(
                            pse[:, qc * 512:(qc + 1) * 512],
                            kt2[0:DH, hp, kt * P:(kt + 1) * P],
                            qt128[0:DH, hp, qc * 512:(qc + 1) * 512],
                            start=True, stop=True,
                        )
                        nc.tensor.matmul(
                            pso[:, qc * 512:(qc + 1) * 512],
                            kt2[DH:P, hp, kt * P:(kt + 1) * P],
                            qt128[DH:P, hp, qc * 512:(qc + 1) * 512],
                            start=True, stop=True, tile_position=(DH, 0),
                        )
                    nc.scalar.activation(ats[0][:, kt, :], pse[:], AF.Exp, scale=SCALE)
                    nc.scalar.activation(ats[1][:, kt, :], pso[:], AF.Exp, scale=SCALE)
                for par in range(2):
                    h = 2 * hp + par
                    at = ats[par]
                    for qc in range(QC):
                        pav = ps.tile([P, 512], F32, tag="mm")
                        for kt in range(KT16):
                            nc.tensor.matmul(
                                pav[0:DH + 1, :],
                                vaug[:, kt, h, :],
                                at[:, kt, qc * 512:(qc + 1) * 512],
                                start=(kt == 0), stop=(kt == KT16 - 1),
                            )
                        ot_sb = scr.tile([DH, 512], F32, tag="otsb", bufs=2)
                        nc.vector.tensor_copy(ot_sb[:], pav[0:DH, :])
                        rd_sb = scr.tile([DH + 1, 512], F32, tag="rds", bufs=2)
                        nc.vector.reciprocal(rd_sb[DH:DH + 1, :], pav[DH:DH + 1, :])
                        rd_sbr = scr.tile([DH + 1, 512], F32R, tag="rdsr", bufs=2)
                        nc.vector.tensor_copy(rd_sbr[DH:DH + 1, :], rd_sb[DH:DH + 1, :])
                        pbc = ps.tile([DH, 512], F32, tag="mm")
                        nc.tensor.matmul(
                            pbc[:], ones2d[DH:DH + 1, :], rd_sbr[DH:DH + 1, :],
                            start=True, stop=True,
                        )
                        nc.vector.tensor_mul(
                            mt_sb[:, h, qc * 512:(qc + 1) * 512], ot_sb[:], pbc[:]
                        )
            pbig_cm.__exit__(None, None, None)
            pb_cm.__exit__(None, None, None)  # free at

            # ================= phase C: mix + residual =================
            for mt in range(DT):
                for qc in range(QC):
                    pm = ps.tile([P, 512], F32, tag="mm")
                    for h in range(H):
                        nc.tensor.matmul(
                            pm[:],
                            wmh_sb[:, h, mt * P:(mt + 1) * P],
                            mt_sb[:, h, qc * 512:(qc + 1) * 512],
                            start=(h == 0), stop=(h == H - 1),
                        )
                    q = qc * 512
                    nc.vector.tensor_add(
                        hxt[:, mt, q:q + 512], pm[:], qt128[:, mt, q:q + 512]
                    )
                    nc.vector.tensor_scalar_add(
                        hxt[:, mt, q:q + 512], hxt[:, mt, q:q + 512], bm_sb[:, mt:mt + 1]
                    )
            pattn_cm.__exit__(None, None, None)  # free qt128/kt2/vaug/mt/wmh

            # ================= phase D: LN1 (feature-major) + FFN =================
            pd_cm = tc.tile_pool(name="pd", bufs=1)
            pd = pd_cm.__enter__()
            pst_cm = tc.tile_pool(name="pst", bufs=2, space="PSUM")
            pst = pst_cm.__enter__()
            w1_sb = pd.tile([P, DT, DFF], BF16, tag="w1")
            nc.gpsimd.dma_start(
                w1_sb[:],
                wall[OFF_W1:OFF_W2].rearrange("(kt p m) -> p kt m", p=P, m=DFF),
            )
            w2_sb = pd.tile([P, FT, D], BF16, tag="w2")
            nc.gpsimd.dma_start(
                w2_sb[:],
                wall[OFF_W2:WTOT].rearrange("(kt p m) -> p kt m", p=P, m=D),
            )

            hxn = pd.tile([P, DT, R], BF16, tag="hxn")
            for qc in range(QC):
                q = qc * 512
                ps_s = pst.tile([1, 512], F32, tag="st")
                for dt in range(DT):
                    nc.tensor.matmul(
                        ps_s[:], onescol[:], hxt[:, dt, q:q + 512],
                        start=(dt == 0), stop=(dt == DT - 1),
                    )
                mean = scr.tile([1, 512], F32, tag="mean", bufs=1)
                nc.vector.tensor_scalar_mul(mean[:], ps_s[:], 1.0 / D)
                ps_q = pst.tile([1, 512], F32, tag="st")
                for dt in range(DT):
                    sqs = scr.tile([P, 512], F32R, tag="sqs", bufs=2)
                    nc.vector.tensor_mul(sqs[:], hxt[:, dt, q:q + 512], hxt[:, dt, q:q + 512])
                    nc.tensor.matmul(
                        ps_q[:], onescol[:], sqs[:],
                        start=(dt == 0), stop=(dt == DT - 1),
                    )
                var = scr.tile([1, 512], F32, tag="lvar", bufs=1)
                nc.vector.tensor_scalar_mul(var[:], ps_q[:], 1.0 / D)
                m2 = scr.tile([1, 512], F32, tag="lm2", bufs=1)
                nc.vector.tensor_mul(m2[:], mean[:], mean[:])
                nc.vector.tensor_sub(var[:], var[:], m2[:])
                nc.vector.tensor_scalar_add(var[:], var[:], EPS)
                std = scr.tile([1, 512], F32, tag="lstd", bufs=1)
                nc.scalar.activation(std[:], var[:], AF.Sqrt)
                rstd32 = scr.tile([1, 512], F32, tag="lrstd32", bufs=1)
                nc.vector.reciprocal(rstd32[:], std[:])
                rstd = scr.tile([1, 512], F32R, tag="lrstd", bufs=1)
                nc.vector.tensor_copy(rstd[:], rstd32[:])
                mrs = scr.tile([1, 512], F32R, tag="lmrs", bufs=1)
                nc.vector.tensor_mul(mrs[:], mean[:], rstd32[:])
                pb_r = ps.tile([P, 512], F32, tag="mm")
                nc.tensor.matmul(pb_r[:], ones1x128[:], rstd[:], start=True, stop=True)
                pb_m = ps.tile([P, 512], F32, tag="mm")
                nc.tensor.matmul(pb_m[:], ones1x128[:], mrs[:], start=True, stop=True)
                for dt in range(DT):
                    nc.vector.tensor_mul(hxn[:, dt, q:q + 512], hxt[:, dt, q:q + 512], pb_r[:])
                    nc.vector.tensor_sub(hxn[:, dt, q:q + 512], hxn[:, dt, q:q + 512], pb_m[:])

            gt = pd.tile([P, FT, R], BF16, tag="gt")
            for ft in range(FT):
                for qc in range(QC):
                    pf = ps.tile([P, 512], F32, tag="mm")
                    for kt in range(DT):
                        nc.tensor.matmul(
                            pf[:],
                            w1_sb[:, kt, ft * P:(ft + 1) * P],
                            hxn[:, kt, qc * 512:(qc + 1) * 512],
                            start=(kt == 0), stop=(kt == DT - 1),
                        )
                    if not gelu_tanh:
                        nc.scalar.activation(
                            gt[:, ft, qc * 512:(qc + 1) * 512], pf[:], AF.Gelu,
                            bias=bb1_sb[:, ft:ft + 1],
                        )
                    else:
                        # sim-only tanh-approx gelu (AF.Gelu unimplemented there)
                        ub = scr.tile([P, 512], F32, tag="gub", bufs=2)
                        nc.scalar.activation(ub[:], pf[:], AF.Identity,
                                             bias=bb1_sb[:, ft:ft + 1])
                        u2 = scr.tile([P, 512], F32, tag="gu2", bufs=2)
                        nc.vector.tensor_mul(u2[:], ub[:], ub[:])
                        nc.vector.tensor_scalar_mul(u2[:], u2[:], 0.044715)
                        nc.vector.tensor_scalar_add(u2[:], u2[:], 1.0)
                        nc.vector.tensor_mul(u2[:], u2[:], ub[:])
                        nc.vector.tensor_scalar_mul(u2[:], u2[:], 0.7978845608028654)
                        th = scr.tile([P, 512], F32, tag="gth", bufs=2)
                        nc.scalar.activation(th[:], u2[:], AF.Tanh)
                        nc.vector.tensor_scalar_add(th[:], th[:], 1.0)
                        nc.vector.tensor_mul(th[:], th[:], ub[:])
                        nc.vector.tensor_scalar_mul(
                            gt[:, ft, qc * 512:(qc + 1) * 512], th[:], 0.5)

            out_sb = pd.tile([P, DT, R], BF16, tag="outsb")
            for mt in range(DT):
                for qc in range(QC):
                    po = ps.tile([P, 512], F32, tag="mm")
                    for kt in range(FT):
                        nc.tensor.matmul(
                            po[:],
                            w2_sb[:, kt, mt * P:(mt + 1) * P],
                            gt[:, kt, qc * 512:(qc + 1) * 512],
                            start=(kt == 0), stop=(kt == FT - 1),
                        )
                    q = qc * 512
                    nc.vector.tensor_add(
                        out_sb[:, mt, q:q + 512], po[:], hxt[:, mt, q:q + 512]
                    )
                    nc.vector.tensor_scalar_add(
                        out_sb[:, mt, q:q + 512], out_sb[:, mt, q:q + 512],
                        bb2_sb[:, mt:mt + 1],
                    )
            nc.gpsimd.dma_start(o_d.ap().rearrange("(mt p) n -> p mt n", p=P), out_sb[:])
            pst_cm.__exit__(None, None, None)
            pd_cm.__exit__(None, None, None)

    nc.compile()
    return nc


def _make_runner(nc, n_cores):
    """Build the reusable jitted SPMD executor for `nc`.

    Mirrors concourse.bass2jax.run_bass_via_pjrt's multi-core branch but
    constructs the jit closure ONCE so repeat calls hit the jit cache
    (run_bass_via_pjrt builds a fresh closure per call, forcing a full
    retrace + executable re-import through the axon tunnel every call).
    """
    import jax
    from jax.sharding import Mesh, PartitionSpec
    from jax.experimental.shard_map import shard_map
    from concourse import bass2jax

    if jax.default_backend() == "axon":
        bass2jax.install_neuronx_cc_hook()

    partition_name = nc.partition_id_tensor.name if nc.partition_id_tensor else None
    in_names, out_names, out_avals, zero_outs = [], [], [], []
    for alloc in nc.m.functions[0].allocations:
        if not isinstance(alloc, mybir.MemoryLocationSet):
            continue
        name = alloc.memorylocations[0].name
        if alloc.kind == "ExternalInput":
            if name != partition_name:
                in_names.append(name)
        elif alloc.kind == "ExternalOutput":
            shape = tuple(alloc.tensor_shape)
            dtype = mybir.dt.np(alloc.dtype)
            out_names.append(name)
            out_avals.append(jax.core.ShapedArray(shape, dtype))
            zero_outs.append(np.zeros((n_cores * shape[0], *shape[1:]), dtype))
    n_params = len(in_names)
    n_outs = len(out_names)
    bind_names = tuple(in_names + out_names + ([partition_name] if partition_name else []))
    donate = tuple(range(n_params, n_params + n_outs))

    def _body(*args):
        operands = list(args)
        if partition_name is not None:
            operands.append(bass2jax.partition_id_tensor())
        outs = bass2jax._bass_exec_p.bind(
            *operands,
            out_avals=tuple(out_avals),
            in_names=bind_names,
            out_names=tuple(out_names),
            lowering_input_output_aliases=(),
            sim_require_finite=True,
            sim_require_nnan=True,
            nc=nc,
        )
        return tuple(outs)

    devices = jax.devices()[:n_cores]
    assert len(devices) == n_cores, f"need {n_cores} devices, have {len(jax.devices())}"
    mesh = Mesh(np.asarray(devices), ("core",))
    in_specs = (PartitionSpec("core"),) * (n_params + n_outs)
    out_specs = (PartitionSpec("core"),) * n_outs
    # cpu (sim) doesn't implement buffer donation; the unaliased
    # jax.buffer_donor annotation trips the sim lowering's check.
    jit_kwargs = {} if jax.default_backend() == "cpu" else dict(donate_argnums=donate)
    fn = jax.jit(
        shard_map(_body, mesh=mesh, in_specs=in_specs, out_specs=out_specs,
                  check_rep=False),
        keep_unused=True, **jit_kwargs,
    )
    sharding = jax.sharding.NamedSharding(mesh, PartitionSpec("core"))
    import jax.numpy as jnp

    zero_shapes = [z.shape for z in zero_outs]
    zero_dtypes = [z.dtype for z in zero_outs]
    zeros_fn = jax.jit(
        lambda: tuple(jnp.zeros(s, d) for s, d in zip(zero_shapes, zero_dtypes)),
        out_shardings=tuple(sharding for _ in zero_outs),
    )
    return dict(fn=fn, in_names=in_names, out_names=out_names,
                out_avals=out_avals, zero_outs=zero_outs, n_cores=n_cores,
                sharding=sharding, zeros_fn=zeros_fn)


def _run_cached(runner, global_in):
    """Run the cached executor on pre-built GLOBAL (concat-over-cores)
    arrays. Inputs are device_put asynchronously in order (host work and
    the serialized tunnel uploads overlap); the donated zero output
    buffers are created on-device (no wire bytes)."""
    import time
    import jax

    n_cores = runner["n_cores"]
    # The terminal-side worker takes minutes to restart after a previous
    # process's teardown; fresh connections see transient
    # NRT_EXEC_UNIT_UNRECOVERABLE / UNAVAILABLE until it's back. Retry
    # patiently — this only ever triggers on the first call of a process.
    for attempt in range(30):
        try:
            dev_in = [jax.device_put(global_in[name], runner["sharding"])
                      for name in runner["in_names"]]
            zeros = runner["zeros_fn"]()
            out_arrs = runner["fn"](*dev_in, *zeros)
            np_out = [np.asarray(a) for a in out_arrs]
            break
        except Exception as e:  # noqa: BLE001
            msg = f"{type(e).__name__}: {e}"
            transient = ("UNRECOVERABLE" in msg or "UNAVAILABLE" in msg
                         or "NRT_" in msg or "PassThrough" in msg)
            if not transient or attempt == 29:
                raise
            time.sleep(40)
    return {
        name: np_out[i].reshape(n_cores, *runner["out_avals"][i].shape)
        for i, name in enumerate(runner["out_names"])
    }


def kernel(X, Y, Wq, bq, Wk, bk, Wv, bv, Wm, bm, g0, b0, g1, b1, W1, bb1, W2, bb2,
           **_ignored):
    X = np.asarray(X, dtype=np.float32)
    Y = np.asarray(Y, dtype=np.float32)
    f32 = lambda a: np.asarray(a, dtype=np.float32)
    Wq, bq, Wk, Wv, bv, Wm, bm = map(f32, (Wq, bq, Wk, Wv, bv, Wm, bm))
    g0, b0, g1, b1, W1, bb1, W2, bb2 = map(f32, (g0, b0, g1, b1, W1, bb1, W2, bb2))

    # host-side exact folds (f32), then quantize the wire copies to bf16.
    # Weights are packed into one flat blob: core c uploads
    # wblob[c*WSH:(c+1)*WSH] and the device AllGathers the full blob —
    # the global (concat-over-cores) wsh array IS wblob, no copies needed.
    # Likewise global x = bf16(X) reshaped, global ysh = fp8(Y^T) raveled.
    gx = X.astype(NPBF16).reshape(NCORES * R, D)
    gy = Y.transpose(0, 2, 1).astype(NPFP8).reshape(NCORES * YSH)
    runner = _cache.get("runner")
    if runner is not None and not _cache.get("run_kwargs"):
        # start the activation uploads now (async) so the fold work below
        # hides under them
        import jax
        gx = jax.device_put(gx, runner["sharding"])
        gy = jax.device_put(gy, runner["sharding"])

    bqv = b0 @ Wq + bq
    bmv = bv @ Wm + bm
    bb1v = b1 @ W1 + bb1
    wblob = np.empty(WTOT, dtype=NPBF16)
    wblob[OFF_WQ:OFF_WK] = (g0[:, None] * Wq).astype(NPBF16).ravel()
    wblob[OFF_WK:OFF_WV] = Wk.astype(NPBF16).ravel()
    wblob[OFF_WV:OFF_WMH] = Wv.astype(NPBF16).ravel()
    wblob[OFF_WMH:OFF_W1] = np.ascontiguousarray(
        Wm.reshape(H, DH, D).transpose(1, 0, 2)).astype(NPBF16).ravel()
    wblob[OFF_W1:OFF_W2] = (g1[:, None] * W1).astype(NPBF16).ravel()
    wblob[OFF_W2:WTOT] = W2.astype(NPBF16).ravel()
    idm = np.eye(P, dtype=NPBF16)
    on1 = np.ones((P, 1), dtype=np.float32)
    on2 = np.ones((1, P), dtype=np.float32)
    onp = np.ones((DH + 1, DH), dtype=np.float32)
    tile8 = lambda a: np.tile(a, (NCORES,) + (1,) * (a.ndim - 1))
    global_in = dict(
        x=gx, ysh=gy, wsh=wblob,
        bq=tile8(bqv), bm=tile8(bmv), bb1=tile8(bb1v), bb2=tile8(bb2),
        idm=tile8(idm), on1=tile8(on1), on2=tile8(on2), onp=tile8(onp),
    )

    if "nc" not in _cache:
        _cache["nc"] = _build(gelu_tanh=_cache.get("gelu_tanh", False))
    nc = _cache["nc"]

    if _cache.get("run_kwargs"):
        in_maps = []
        for c in range(NCORES):
            b, half = c // 2, c % 2
            in_maps.append(dict(
                x=np.ascontiguousarray(global_in["x"][c * R:(c + 1) * R]),
                ysh=np.ascontiguousarray(global_in["ysh"][c * YSH:(c + 1) * YSH]),
                wsh=np.ascontiguousarray(wblob[c * WSH:(c + 1) * WSH]),
                bq=bqv, bm=bmv, bb1=bb1v, bb2=bb2, idm=idm,
                on1=on1, on2=on2, onp=onp,
            ))
        res = run_bass_kernel_spmd(nc, in_maps, core_ids=list(range(NCORES)),
                                   **_cache["run_kwargs"])
        _cache["last"] = res
        o_g = np.stack([res.results[c]["o"] for c in range(NCORES)])
    else:
        if "runner" not in _cache:
            _cache["runner"] = _make_runner(nc, NCORES)
        o_g = _run_cached(_cache["runner"], global_in)["o"]
        _cache["last"] = None
    # o_g: [8, D, R] bf16, core c = (batch c//2, row-half c%2)
    out = o_g.transpose(0, 2, 1).astype(np.float32).reshape(B, N, D)
    return out
